# revision 1
# baseline (speedup 1.0000x reference)
"""Causal self-attention on 8 TRN2 NeuronCores, batch-data-parallel (one batch
element per core).

Layout strategy (per core, S=1024, D=1024, H=16, hd=64):
  - Host pre-transposes x -> xT [D,S] and all weights -> [in_dim, out_dim].
  - qk projection produces q,k transposed ([e,s]) per head-pair: lhsT = wqkT
    tiles, rhs = xT.  Head h lives at partitions 64*(h%2)..+64.
  - v natural [s,e]: lhsT = xT tiles, rhs = wvT tiles; stored interleaved with
    a ones column per head (65 cols/head) so the AV matmul's PSUM row 64 is
    the softmax denominator (rowsum of unnormalized attn).
  - scoresT [sk,sq] per head-pair via row-group-packed K=64 matmuls; exp on
    ACT (scale=1/8 folded in); causal diag masked by multiplicative
    upper-triangular mask; fully-masked tiles never computed.
  - AV: outT'[hd+1, sq] accumulated m-major in 512-wide chunks; normalization
    via approx-reciprocal of the den row + PE rank-1 broadcast + DVE multiply.
  - proj: y[s,e] with lhsT = outT tiles, rhs = wpT tiles + rank-1 bias term
    (beff = b_proj + W_proj @ b_v; b_v folds exactly through softmax rowsum).
  - QKV matmul quanta are interleaved into the attention pair loop so the PE
    stream stays dense while ACT runs exp (keeps the HAM clock gate at 8/8).
All matmuls run in float32r (TF32-like, full PE rate at N>=256).
"""

import numpy as np

B, S, D, H = 8, 1024, 1024, 16
HD = D // H          # 64
P = 128
NCORES = 8
KO = D // P          # 8 contraction tiles over d
MT = (2 * D) // P    # 16 m-tiles for q,k
ST = S // P          # 8 s-tiles
NPAIRS = H // 2      # 8 head pairs

_CACHE = {}
TRACE = False        # set by test harness to collect an NTFF profile


def _score_chunks(w):
    # split w into pieces, each >=256 when possible (fp32r full-rate needs
    # moving dim >=256), <=512 (PSUM bank limit)
    table = {1024: [512, 512], 896: [512, 384], 768: [512, 256],
             640: [384, 256], 512: [512], 384: [384], 256: [256], 128: [128]}
    return table[w]


def _build():
    import concourse.tile as tile
    from concourse import bacc, mybir

    F32R = mybir.dt.float32r
    F32 = mybir.dt.float32
    AF = mybir.ActivationFunctionType

    nc = bacc.Bacc("TRN2", target_bir_lowering=False, debug=False,
                   num_devices=NCORES)
    xT_d = nc.dram_tensor("xT", [D, S], F32R, kind="ExternalInput").ap()
    wqkT_d = nc.dram_tensor("wqkT", [D, 2 * D], F32R, kind="ExternalInput").ap()
    wvT_d = nc.dram_tensor("wvT", [D, D], F32R, kind="ExternalInput").ap()
    wpT_d = nc.dram_tensor("wpT", [D, D], F32R, kind="ExternalInput").ap()
    bqk_d = nc.dram_tensor("bqk", [2 * D], F32, kind="ExternalInput").ap()
    beff_d = nc.dram_tensor("beff", [D], F32R, kind="ExternalInput").ap()
    umask_d = nc.dram_tensor("umask", [P, P], F32, kind="ExternalInput").ap()
    y_d = nc.dram_tensor("y", [S, D], F32, kind="ExternalOutput").ap()

    wqkT_v = wqkT_d.rearrange("(ko p) e -> p ko e", p=P)
    wvT_v = wvT_d.rearrange("(ko p) e -> p ko e", p=P)
    wpT_v = wpT_d.rearrange("(ko p) e -> p ko e", p=P)
    xT_v = xT_d.rearrange("(ko p) s -> p ko s", p=P)

    with tile.TileContext(nc) as tc:
        with (
            tc.tile_pool(name="bigio", bufs=1) as bigio,
            tc.tile_pool(name="qkp", bufs=3) as qkp,
            tc.tile_pool(name="vp", bufs=1) as vpool,
            tc.tile_pool(name="wqk", bufs=2) as wqkp,
            tc.tile_pool(name="wk1", bufs=3) as wk1,
            tc.tile_pool(name="attn", bufs=5) as attnp,
            tc.tile_pool(name="rt", bufs=2) as rtp,
            tc.tile_pool(name="rb", bufs=1) as rbp,
            tc.tile_pool(name="todd", bufs=1) as toddp,
            tc.tile_pool(name="ystg", bufs=1) as ystgp,
            tc.tile_pool(name="avsb", bufs=2) as avsbp,
            tc.tile_pool(name="cst", bufs=1) as cst,
            tc.tile_pool(name="psS", bufs=4, space="PSUM") as psS,
            tc.tile_pool(name="psAV", bufs=2, space="PSUM") as psAV,
        ):
            # ---------- constants ----------
            umask = cst.tile([P, P], F32)
            nc.sync.dma_start(umask[:], umask_d)
            bqk_sb = cst.tile([P, MT], F32)
            nc.sync.dma_start(bqk_sb[:], bqk_d.rearrange("(m p) -> p m", p=P))
            beff_sb = cst.tile([1, D], F32R)
            nc.sync.dma_start(beff_sb[:], beff_d[None, :])
            onecol = cst.tile([P, 1], F32)
            nc.vector.memset(onecol[:], 1.0)
            ones1x128 = cst.tile([1, P], F32R)
            nc.vector.tensor_copy(
                ones1x128[:], onecol[0:1, :].broadcast_to([1, P]))
            of32 = cst.tile([65, 64], F32)
            nc.vector.memset(of32[64:65, :], 1.0)
            ones65r = cst.tile([65, 64], F32R)
            nc.vector.tensor_copy(ones65r[64:65, :], of32[64:65, :])
            zrow = cst.tile([P, 384], F32)
            nc.vector.memset(zrow[:], 0.0)

            # ---------- big SBUF residents ----------
            xT = bigio.tile([P, KO, S], F32R, tag="xT")
            for ko in range(KO):
                nc.sync.dma_start(xT[:, ko, :], xT_v[:, ko, :])
            outT = bigio.tile([P, KO, S], F32R, tag="outT")
            v_sb = vpool.tile([P, ST, H * (HD + 1)], F32R)
            v_hview = v_sb[:].rearrange("p st (h c) -> p st h c", c=HD + 1)
            nc.vector.tensor_copy(
                v_hview[:, :, :, HD:HD + 1],
                onecol[:, None, None, :].broadcast_to([P, ST, H, 1]))

            qk_tiles = {}    # j -> [128, 2, S] tile (0=q, 1=k)

            # ---------- QKV work quanta (emitted interleaved) ----------
            def qk_quanta(j):
                # 4 closures; each computes one (part, nn) psum group
                t = qkp.tile([P, 2, S], F32R, tag="qkt", name=f"qk{j}")
                qk_tiles[j] = t
                wts = {}

                def quantum(part, nn):    # part 0=q (m-tile j), 1=k (8+j)
                    def go():
                        m = j if part == 0 else NPAIRS + j
                        if part not in wts:
                            wt = wqkp.tile([P, KO, P], F32R, tag="wqk", name=f"wqk{m}")
                            nc.sync.dma_start(
                                wt[:], wqkT_v[:, :, m * P:(m + 1) * P])
                            wts[part] = wt
                        wt = wts[part]
                        ps = psS.tile([P, 512], F32, tag="ps", name=f"qkps{m}")
                        for ko in range(KO):
                            nc.tensor.matmul(
                                ps[:], wt[:, ko, :],
                                xT[:, ko, nn * 512:(nn + 1) * 512],
                                start=(ko == 0), stop=(ko == KO - 1))
                        nc.vector.tensor_scalar_add(
                            t[:, part, nn * 512:(nn + 1) * 512], ps[:],
                            bqk_sb[:, m:m + 1])
                    return go
                return [quantum(0, 0), quantum(0, 1),
                        quantum(1, 0), quantum(1, 1)]

            def v_quanta(nE):
                # v half nE: e_v cols 512*nE.. (heads 8nE..8nE+7), 4 quanta
                # of 2 s-tiles; weights DMA'd in [128, 2, 512] blocks so the
                # PE never starves on small weight transfers
                def quantum(g0):
                    def go():
                        sts = [g0, g0 + 1]
                        pss = {}
                        for st in sts:
                            pss[st] = psS.tile([P, 512], F32, tag="ps",
                                               name=f"vps{nE}_{st}")
                        for kog in range(KO // 2):
                            wv_t = wk1.tile([P, 2, 512], F32R, tag="wk1",
                                            name=f"wv{nE}_{kog}")
                            nc.sync.dma_start(
                                wv_t[:],
                                wvT_v[:, 2 * kog:2 * kog + 2,
                                      nE * 512:(nE + 1) * 512])
                            for k2 in range(2):
                                ko = 2 * kog + k2
                                for st in sts:
                                    nc.tensor.matmul(
                                        pss[st][:],
                                        xT[:, ko, st * P:(st + 1) * P],
                                        wv_t[:, k2, :], start=(ko == 0),
                                        stop=(ko == KO - 1))
                        for st in sts:
                            nc.vector.tensor_copy(
                                v_hview[:, st, 8 * nE:8 * (nE + 1), 0:HD],
                                pss[st][:].rearrange("p (h c) -> p h c", c=HD))
                    return go
                return [quantum(g) for g in (0, 2, 4, 6)]

            # ---------- attention ----------
            pend = {}

            def scores_exp(j, m):
                qk_t = qk_tiles[j]
                w = S - m * P
                for hb, base in ((0, 0), (1, 64)):   # head 2j+hb
                    at = attnp.tile([P, S], F32R, tag="at",
                                    name=f"at{j}_{hb}_{m}")
                    pend[(j, hb, m)] = at
                    gw = m * P - (0 if m <= 3 else 512)
                    if 0 < gw < 512:
                        nc.vector.tensor_copy(
                            at[:, m * P - gw:m * P], zrow[:, 0:gw])
                    off = m * P
                    for cw in _score_chunks(w):
                        ps = psS.tile([P, 512], F32, tag="ps",
                                      name=f"sps{j}_{hb}_{m}")
                        nc.tensor.matmul(
                            ps[:, 0:cw],
                            qk_t[base:base + 64, 1, m * P:(m + 1) * P],
                            qk_t[base:base + 64, 0, off:off + cw],
                            start=True, stop=True)
                        nc.scalar.activation(
                            at[:, off:off + cw], ps[:, 0:cw], AF.Exp,
                            scale=0.125)
                        off += cw
                    nc.vector.tensor_mul(
                        at[:, m * P:(m + 1) * P], at[:, m * P:(m + 1) * P],
                        umask[:])

            def av_m(j, m):
                st8 = pend[f"ps{j}"]
                for hb in (0, 1):
                    h = 2 * j + hb
                    at = pend[(j, hb, m)]
                    for n in range((0 if m <= 3 else 1), 2):
                        nc.tensor.matmul(
                            st8[hb][:, n * 512:(n + 1) * 512],
                            v_sb[:, m, h * (HD + 1):(h + 1) * (HD + 1)],
                            at[:, n * 512:(n + 1) * 512],
                            start=(m == 0), stop=(m == 4 * n + 3))

            def evict_recip(j):
                # move the [65, S] AV accumulators out of PSUM so the next
                # pair's AV matmuls get the PSUM slots immediately, then take
                # the reciprocal of the den row (approx_fast: ~4e-6 rel, 5x
                # faster than exact; den >= exp(0) > 0 so no edge cases)
                avcs, recs = [], []
                for hb in (0, 1):
                    avc = avsbp.tile([65, S], F32R, tag="avc",
                                     name=f"avc{j}_{hb}")
                    nc.vector.tensor_copy(avc[:], pend[f"ps{j}"][hb][:])
                    avcs.append(avc)
                    rt = rtp.tile([65, S], F32R, tag="rt")
                    rt32 = rtp.tile([65, S], F32, tag="rt32", bufs=1)
                    # custom-DVE op misbehaves on single-partition APs on HW:
                    # run it over all 65 rows (lanes are parallel) and consume
                    # only the den row (64); other lanes are never read
                    nc.vector.reciprocal_approx_fast(
                        rt32[:], avc[:].bitcast(F32))
                    nc.vector.tensor_copy(rt[64:65, :], rt32[64:65, :])
                    recs.append(rt)
                pend[f"avc{j}"] = avcs
                pend[f"rec{j}"] = recs
                del pend[f"ps{j}"]

            def rb_norm(j):
                for hb in (0, 1):
                    rt = pend[f"rec{j}"][hb]
                    rb_t = rbp.tile([64, S], F32R, tag="rb")
                    for c in range(2):
                        rps = psS.tile([P, 512], F32, tag="ps",
                                       name=f"rbps{j}_{hb}_{c}")
                        nc.tensor.matmul(
                            rps[0:64, :], ones65r[64:65, :],
                            rt[64:65, c * 512:(c + 1) * 512],
                            start=True, stop=True)
                        nc.vector.tensor_copy(
                            rb_t[:, c * 512:(c + 1) * 512], rps[0:64, :])
                    avc = pend[f"avc{j}"][hb]
                    if hb == 0:
                        nc.vector.tensor_mul(
                            outT[0:64, j, :], avc[0:64, :], rb_t[:])
                    else:
                        # DVE lanes cannot shift partitions: multiply to an
                        # SBUF tmp, then DMA-shift rows 0..63 -> 64..127
                        tmp = toddp.tile([64, S], F32R, tag="todd")
                        nc.vector.tensor_mul(tmp[:], avc[0:64, :], rb_t[:])
                        nc.sync.dma_start(outT[64:128, j, :], tmp[:])
                del pend[f"avc{j}"], pend[f"rec{j}"]

            # ---------- interleaved emission ----------
            # prologue: qk for pairs 0,1 and v half 0
            for q in qk_quanta(0):
                q()
            for q in qk_quanta(1):
                q()
            for q in v_quanta(0):
                q()
            vwork = list(v_quanta(1))   # needed from pair 4 on

            for j in range(NPAIRS):
                # qkv work to interleave into this pair's m-steps
                work = []
                if j + 2 < NPAIRS:
                    work.extend(qk_quanta(j + 2))
                if j < 3 and vwork:
                    work.append(vwork.pop(0))
                    if j == 2:
                        work.append(vwork.pop(0))
                for m in range(ST):
                    scores_exp(j, m)
                    if m == 4 and j > 0:
                        rb_norm(j - 1)
                    if m == 0:
                        pend[f"ps{j}"] = [
                            psAV.tile([65, S], F32, tag="av",
                                      name=f"av{j}_{hb}") for hb in range(2)]
                    if m >= 2:
                        av_m(j, m - 2)
                    if m % 2 == 1 and work:
                        work.pop(0)()
                        if work and j % 2 == 0:
                            work.pop(0)()
                av_m(j, ST - 2)
                while work:
                    work.pop(0)()
                av_m(j, ST - 1)
                evict_recip(j)
            rb_norm(NPAIRS - 1)

            # ---------- output projection ----------
            for g0 in range(0, ST, 3):
                sts = list(range(g0, min(g0 + 3, ST)))
                for nE in range(2):
                    pss = {st: psS.tile([P, 512], F32, tag="ps",
                                        name=f"yps{st}") for st in sts}
                    for kog in range(KO // 2):
                        wp_t = wk1.tile([P, 2, 512], F32R, tag="wk1",
                                        name=f"wp{kog}_{nE}")
                        nc.sync.dma_start(
                            wp_t[:], wpT_v[:, 2 * kog:2 * kog + 2,
                                           nE * 512:(nE + 1) * 512])
                        for k2 in range(2):
                            ko = 2 * kog + k2
                            for st in sts:
                                nc.tensor.matmul(
                                    pss[st][:],
                                    outT[:, ko, st * P:(st + 1) * P],
                                    wp_t[:, k2, :], start=(ko == 0),
                                    stop=False)
                    for st in sts:
                        nc.tensor.matmul(
                            pss[st][:], ones1x128[:],
                            beff_sb[:, nE * 512:(nE + 1) * 512],
                            start=False, stop=True)
                        ystg = ystgp.tile([P, 512], F32, tag="ystg",
                                          name=f"ystg{st}")
                        nc.vector.tensor_copy(ystg[:], pss[st][:])
                        nc.sync.dma_start(
                            y_d[st * P:(st + 1) * P, nE * 512:(nE + 1) * 512],
                            ystg[:])

    nc.compile()
    return nc


def kernel(x, w_attn, b_attn, w_proj, b_proj):
    import concourse.bass_utils as bass_utils

    if "nc" not in _CACHE:
        _CACHE["nc"] = _build()
    nc = _CACHE["nc"]

    x = np.asarray(x, dtype=np.float32)
    w_attn = np.asarray(w_attn, dtype=np.float32)
    b_attn = np.asarray(b_attn, dtype=np.float32)
    w_proj = np.asarray(w_proj, dtype=np.float32)
    b_proj = np.asarray(b_proj, dtype=np.float32)

    xT = np.ascontiguousarray(np.transpose(x, (0, 2, 1)))        # [B, D, S]
    wqkT = np.ascontiguousarray(w_attn[:2 * D].T)                # [D, 2D]
    wvT = np.ascontiguousarray(w_attn[2 * D:].T)                 # [D, D]
    wpT = np.ascontiguousarray(w_proj.T)                         # [D, D]
    bqk = np.ascontiguousarray(b_attn[:2 * D])
    bv = b_attn[2 * D:]
    beff = (b_proj.astype(np.float64)
            + w_proj.astype(np.float64) @ bv.astype(np.float64)
            ).astype(np.float32)
    umask = np.triu(np.ones((P, P), dtype=np.float32))           # f >= p

    in_maps = [
        dict(xT=xT[b], wqkT=wqkT, wvT=wvT, wpT=wpT, bqk=bqk, beff=beff,
             umask=umask)
        for b in range(B)
    ]
    res = bass_utils.run_bass_kernel_spmd(
        nc, in_maps, core_ids=list(range(NCORES)), trace=TRACE)
    if TRACE:
        _CACHE["exec_time_ns"] = res.exec_time_ns
        _CACHE["trace"] = res.instructions_and_trace
    return np.stack([res.results[b]["y"] for b in range(B)], axis=0)



# revision 3
# speedup vs baseline: 1.2826x; 1.2826x over previous
"""Causal self-attention on 8 TRN2 NeuronCores, batch-data-parallel (one batch
element per core).

Layout strategy (per core, S=1024, D=1024, H=16, hd=64):
  - Host pre-transposes x -> xT [D,S] and all weights -> [in_dim, out_dim],
    and pre-rounds all matmul operands to bf16 (PE runs bf16 at 1 col/cycle
    like fp32r, but at roughly half the power -> far less HAM clock-gate
    throttling; DVE gets 2x on 16-bit; DMA volume halves).
  - qk projection produces q,k transposed ([e,s]) per head-pair: lhsT = wqkT
    tiles, rhs = xT.  Head h lives at partitions 64*(h%2)..+64.
  - v natural [s,e]: lhsT = xT tiles, rhs = wvT tiles; stored interleaved with
    a ones column per head (65 cols/head) so the AV matmul's PSUM row 64 is
    the softmax denominator (rowsum of unnormalized attn).
  - scoresT [sk,sq] per head-pair via K=64 matmuls; exp on ACT (scale=1/8
    folded in, bf16 out); causal diag masked by multiplicative
    upper-triangular mask; fully-masked tiles never computed.
  - AV: outT'[hd+1, sq] accumulated m-major in 512-wide chunks; normalization
    via approx-reciprocal of the den row + PE rank-1 broadcast + DVE multiply.
  - proj: y[s,e] with lhsT = outT tiles, rhs = resident wpT (prefetched during
    attention so the tail never waits on DMA) + rank-1 bias term
    (beff = b_proj + W_proj @ b_v; b_v folds exactly through softmax rowsum).
  - QKV matmul quanta are interleaved into the attention pair loop so the PE
    stream stays dense while ACT runs exp (keeps the HAM clock gate at 8/8).
  - xT is split into 8 per-ko tiles so the first QKV matmuls only wait on the
    first 256KB of DMA, not the full 2MB.
"""

import numpy as np

B, S, D, H = 8, 1024, 1024, 16
HD = D // H          # 64
P = 128
NCORES = 8
KO = D // P          # 8 contraction tiles over d
MT = (2 * D) // P    # 16 m-tiles for q,k
ST = S // P          # 8 s-tiles
NPAIRS = H // 2      # 8 head pairs

_CACHE = {}
TRACE = False        # set by test harness to collect an NTFF profile


def _score_chunks(w):
    # split w into pieces <=512 (PSUM bank limit); bf16 streams at full rate
    # at any width so no >=256 constraint
    table = {1024: [512, 512], 896: [512, 384], 768: [512, 256],
             640: [384, 256], 512: [512], 384: [384], 256: [256], 128: [128]}
    return table[w]


def _build():
    import concourse.tile as tile
    from concourse import bacc, mybir

    F32R = mybir.dt.float32r
    F32 = mybir.dt.float32
    BF16 = mybir.dt.bfloat16
    AF = mybir.ActivationFunctionType

    nc = bacc.Bacc("TRN2", target_bir_lowering=False, debug=False,
                   num_devices=NCORES)
    xT_d = nc.dram_tensor("xT", [D, S], BF16, kind="ExternalInput").ap()
    wqkT_d = nc.dram_tensor("wqkT", [D, 2 * D], BF16, kind="ExternalInput").ap()
    wvT_d = nc.dram_tensor("wvT", [D, D], BF16, kind="ExternalInput").ap()
    wpT_d = nc.dram_tensor("wpT", [D, D], BF16, kind="ExternalInput").ap()
    bqk_d = nc.dram_tensor("bqk", [2 * D], F32, kind="ExternalInput").ap()
    beff_d = nc.dram_tensor("beff", [D], BF16, kind="ExternalInput").ap()
    umask_d = nc.dram_tensor("umask", [P, P], BF16, kind="ExternalInput").ap()
    y_d = nc.dram_tensor("y", [S, D], F32, kind="ExternalOutput").ap()

    wqkT_v = wqkT_d.rearrange("(ko p) e -> p ko e", p=P)
    wvT_v = wvT_d.rearrange("(ko p) e -> p ko e", p=P)
    wpT_v = wpT_d.rearrange("(ko p) e -> p ko e", p=P)
    xT_v = xT_d.rearrange("(ko p) s -> p ko s", p=P)

    with tile.TileContext(nc) as tc:
        with (
            tc.tile_pool(name="bigio", bufs=1) as bigio,
            tc.tile_pool(name="qkp", bufs=3) as qkp,
            tc.tile_pool(name="vp", bufs=1) as vpool,
            tc.tile_pool(name="wqk", bufs=2) as wqkp,
            tc.tile_pool(name="wk1", bufs=3) as wk1,
            tc.tile_pool(name="attn", bufs=8) as attnp,
            tc.tile_pool(name="rt", bufs=2) as rtp,
            tc.tile_pool(name="rb", bufs=1) as rbp,
            tc.tile_pool(name="todd", bufs=1) as toddp,
            tc.tile_pool(name="ystg", bufs=2) as ystgp,
            tc.tile_pool(name="avsb", bufs=2) as avsbp,
            tc.tile_pool(name="cst", bufs=1) as cst,
            tc.tile_pool(name="psS", bufs=4, space="PSUM") as psS,
            tc.tile_pool(name="psAV", bufs=2, space="PSUM") as psAV,
        ):
            # ---------- constants ----------
            umask = cst.tile([P, P], BF16)
            nc.sync.dma_start(umask[:], umask_d)
            bqk_sb = cst.tile([P, MT], F32)
            nc.sync.dma_start(bqk_sb[:], bqk_d.rearrange("(m p) -> p m", p=P))
            beff_sb = cst.tile([1, D], BF16)
            nc.sync.dma_start(beff_sb[:], beff_d[None, :])
            onecol = cst.tile([P, 1], BF16)
            nc.vector.memset(onecol[:], 1.0)
            ones1x128 = cst.tile([1, P], BF16)
            nc.vector.tensor_copy(
                ones1x128[:], onecol[0:1, :].broadcast_to([1, P]))
            of32 = cst.tile([65, 64], F32)
            nc.vector.memset(of32[64:65, :], 1.0)
            ones65r = cst.tile([65, 64], F32R)
            nc.vector.tensor_copy(ones65r[64:65, :], of32[64:65, :])
            zrow = cst.tile([P, 384], BF16)
            nc.vector.memset(zrow[:], 0.0)

            # ---------- big SBUF residents ----------
            xT = [bigio.tile([P, S], BF16, tag=f"xT{ko}", name=f"xT{ko}")
                  for ko in range(KO)]
            for ko in range(KO):
                nc.sync.dma_start(xT[ko][:], xT_v[:, ko, :])
            outT = bigio.tile([P, KO, S], BF16, tag="outT")
            wp_sb = bigio.tile([P, KO, D], BF16, tag="wp")
            v_sb = vpool.tile([P, ST, H * (HD + 1)], BF16)
            v_hview = v_sb[:].rearrange("p st (h c) -> p st h c", c=HD + 1)
            nc.vector.tensor_copy(
                v_hview[:, :, :, HD:HD + 1],
                onecol[:, None, None, :].broadcast_to([P, ST, H, 1]))

            qk_tiles = {}    # j -> [128, 2, S] tile (0=q, 1=k)

            # ---------- QKV work quanta (emitted interleaved) ----------
            def qk_quanta(j):
                # 4 closures; each computes one (part, nn) psum group
                t = qkp.tile([P, 2, S], BF16, tag="qkt", name=f"qk{j}")
                qk_tiles[j] = t
                wts = {}

                def quantum(part, nn):    # part 0=q (m-tile j), 1=k (8+j)
                    def go():
                        m = j if part == 0 else NPAIRS + j
                        if part not in wts:
                            wt = wqkp.tile([P, KO, P], BF16, tag="wqk", name=f"wqk{m}")
                            nc.sync.dma_start(
                                wt[:], wqkT_v[:, :, m * P:(m + 1) * P])
                            wts[part] = wt
                        wt = wts[part]
                        ps = psS.tile([P, 512], F32, tag="ps", name=f"qkps{m}")
                        for ko in range(KO):
                            nc.tensor.matmul(
                                ps[:], wt[:, ko, :],
                                xT[ko][:, nn * 512:(nn + 1) * 512],
                                start=(ko == 0), stop=(ko == KO - 1))
                        nc.vector.tensor_scalar_add(
                            t[:, part, nn * 512:(nn + 1) * 512], ps[:],
                            bqk_sb[:, m:m + 1])
                    return go
                return [quantum(0, 0), quantum(0, 1),
                        quantum(1, 0), quantum(1, 1)]

            def v_quanta(nE):
                # v half nE: e_v cols 512*nE.. (heads 8nE..8nE+7), 4 quanta
                # of 2 s-tiles; weights DMA'd in [128, 2, 512] blocks so the
                # PE never starves on small weight transfers
                def quantum(g0):
                    def go():
                        sts = [g0, g0 + 1]
                        pss = {}
                        for st in sts:
                            pss[st] = psS.tile([P, 512], F32, tag="ps",
                                               name=f"vps{nE}_{st}")
                        for kog in range(KO // 2):
                            wv_t = wk1.tile([P, 2, 512], BF16, tag="wk1",
                                            name=f"wv{nE}_{kog}")
                            nc.sync.dma_start(
                                wv_t[:],
                                wvT_v[:, 2 * kog:2 * kog + 2,
                                      nE * 512:(nE + 1) * 512])
                            for k2 in range(2):
                                ko = 2 * kog + k2
                                for st in sts:
                                    nc.tensor.matmul(
                                        pss[st][:],
                                        xT[ko][:, st * P:(st + 1) * P],
                                        wv_t[:, k2, :], start=(ko == 0),
                                        stop=(ko == KO - 1))
                        for st in sts:
                            nc.vector.tensor_copy(
                                v_hview[:, st, 8 * nE:8 * (nE + 1), 0:HD],
                                pss[st][:].rearrange("p (h c) -> p h c", c=HD))
                    return go
                return [quantum(g) for g in (0, 2, 4, 6)]

            # ---------- attention ----------
            pend = {}

            def scores_exp(j, m):
                qk_t = qk_tiles[j]
                w = S - m * P
                for hb, base in ((0, 0), (1, 64)):   # head 2j+hb
                    at = attnp.tile([P, S], BF16, tag="at",
                                    name=f"at{j}_{hb}_{m}")
                    pend[(j, hb, m)] = at
                    gw = m * P - (0 if m <= 3 else 512)
                    if 0 < gw < 512:
                        nc.vector.tensor_copy(
                            at[:, m * P - gw:m * P], zrow[:, 0:gw])
                    off = m * P
                    for cw in _score_chunks(w):
                        ps = psS.tile([P, 512], F32, tag="ps",
                                      name=f"sps{j}_{hb}_{m}")
                        nc.tensor.matmul(
                            ps[:, 0:cw],
                            qk_t[base:base + 64, 1, m * P:(m + 1) * P],
                            qk_t[base:base + 64, 0, off:off + cw],
                            start=True, stop=True)
                        nc.scalar.activation(
                            at[:, off:off + cw], ps[:, 0:cw], AF.Exp,
                            scale=0.125)
                        off += cw
                    nc.vector.tensor_mul(
                        at[:, m * P:(m + 1) * P], at[:, m * P:(m + 1) * P],
                        umask[:])

            def av_m(j, m):
                st8 = pend[f"ps{j}"]
                for hb in (0, 1):
                    h = 2 * j + hb
                    at = pend[(j, hb, m)]
                    for n in range((0 if m <= 3 else 1), 2):
                        nc.tensor.matmul(
                            st8[hb][:, n * 512:(n + 1) * 512],
                            v_sb[:, m, h * (HD + 1):(h + 1) * (HD + 1)],
                            at[:, n * 512:(n + 1) * 512],
                            start=(m == 0), stop=(m == 4 * n + 3))

            def evict_recip(j):
                # move the [65, S] AV accumulators out of PSUM so the next
                # pair's AV matmuls get the PSUM slots immediately, then take
                # the reciprocal of the den row (approx_fast: ~4e-6 rel, 5x
                # faster than exact; den >= exp(0) > 0 so no edge cases)
                avcs, recs = [], []
                for hb in (0, 1):
                    avc = avsbp.tile([65, S], F32R, tag="avc",
                                     name=f"avc{j}_{hb}")
                    nc.vector.tensor_copy(avc[:], pend[f"ps{j}"][hb][:])
                    avcs.append(avc)
                    rt = rtp.tile([65, S], F32R, tag="rt")
                    rt32 = rtp.tile([65, S], F32, tag="rt32", bufs=1)
                    # custom-DVE op misbehaves on single-partition APs on HW:
                    # run it over all 65 rows (lanes are parallel) and consume
                    # only the den row (64); other lanes are never read
                    nc.vector.reciprocal_approx_fast(
                        rt32[:], avc[:].bitcast(F32))
                    nc.vector.tensor_copy(rt[64:65, :], rt32[64:65, :])
                    recs.append(rt)
                pend[f"avc{j}"] = avcs
                pend[f"rec{j}"] = recs
                del pend[f"ps{j}"]

            def rb_norm(j):
                for hb in (0, 1):
                    rt = pend[f"rec{j}"][hb]
                    rb_t = rbp.tile([64, S], F32R, tag="rb")
                    for c in range(2):
                        rps = psS.tile([P, 512], F32, tag="ps",
                                       name=f"rbps{j}_{hb}_{c}")
                        nc.tensor.matmul(
                            rps[0:64, :], ones65r[64:65, :],
                            rt[64:65, c * 512:(c + 1) * 512],
                            start=True, stop=True)
                        nc.vector.tensor_copy(
                            rb_t[:, c * 512:(c + 1) * 512], rps[0:64, :])
                    avc = pend[f"avc{j}"][hb]
                    if hb == 0:
                        nc.vector.tensor_mul(
                            outT[0:64, j, :], avc[0:64, :], rb_t[:])
                    else:
                        # DVE lanes cannot shift partitions: multiply to an
                        # SBUF tmp, then DMA-shift rows 0..63 -> 64..127
                        tmp = toddp.tile([64, S], BF16, tag="todd")
                        nc.vector.tensor_mul(tmp[:], avc[0:64, :], rb_t[:])
                        nc.sync.dma_start(outT[64:128, j, :], tmp[:])
                del pend[f"avc{j}"], pend[f"rec{j}"]

            # ---------- interleaved emission ----------
            # prologue: qk for pairs 0,1 and v half 0
            for q in qk_quanta(0):
                q()
            # prefetch the full projection weight while DMA engines are idle
            # (used only at the very end; being resident kills the tail's
            # 3-5us DMA-wait gaps)
            for ko in range(KO):
                nc.sync.dma_start(wp_sb[:, ko, :], wpT_v[:, ko, :])
            for q in qk_quanta(1):
                q()
            for q in v_quanta(0):
                q()
            vwork = list(v_quanta(1))   # needed from pair 4 on

            for j in range(NPAIRS):
                # qkv work to interleave into this pair's m-steps
                work = []
                if j + 2 < NPAIRS:
                    work.extend(qk_quanta(j + 2))
                if j < 3 and vwork:
                    work.append(vwork.pop(0))
                    if j == 2:
                        work.append(vwork.pop(0))
                for m in range(ST):
                    scores_exp(j, m)
                    if m == 4 and j > 0:
                        rb_norm(j - 1)
                    if m == 0:
                        pend[f"ps{j}"] = [
                            psAV.tile([65, S], F32, tag="av",
                                      name=f"av{j}_{hb}") for hb in range(2)]
                    if m >= 2:
                        av_m(j, m - 2)
                    if m % 2 == 1 and work:
                        work.pop(0)()
                        if work and j % 2 == 0:
                            work.pop(0)()
                av_m(j, ST - 2)
                while work:
                    work.pop(0)()
                av_m(j, ST - 1)
                evict_recip(j)
            rb_norm(NPAIRS - 1)

            # ---------- output projection ----------
            for g0 in range(0, ST, 3):
                sts = list(range(g0, min(g0 + 3, ST)))
                for nE in range(2):
                    pss = {st: psS.tile([P, 512], F32, tag="ps",
                                        name=f"yps{st}") for st in sts}
                    for ko in range(KO):
                        for st in sts:
                            nc.tensor.matmul(
                                pss[st][:],
                                outT[:, ko, st * P:(st + 1) * P],
                                wp_sb[:, ko, nE * 512:(nE + 1) * 512],
                                start=(ko == 0), stop=False)
                    for st in sts:
                        nc.tensor.matmul(
                            pss[st][:], ones1x128[:],
                            beff_sb[:, nE * 512:(nE + 1) * 512],
                            start=False, stop=True)
                        ystg = ystgp.tile([P, 512], F32, tag="ystg",
                                          name=f"ystg{st}")
                        nc.vector.tensor_copy(ystg[:], pss[st][:])
                        nc.sync.dma_start(
                            y_d[st * P:(st + 1) * P, nE * 512:(nE + 1) * 512],
                            ystg[:])

    nc.compile()
    return nc


def kernel(x, w_attn, b_attn, w_proj, b_proj):
    import concourse.bass_utils as bass_utils
    import ml_dtypes

    if "nc" not in _CACHE:
        _CACHE["nc"] = _build()
    nc = _CACHE["nc"]

    BF = ml_dtypes.bfloat16
    x = np.asarray(x, dtype=np.float32)
    w_attn = np.asarray(w_attn, dtype=np.float32)
    b_attn = np.asarray(b_attn, dtype=np.float32)
    w_proj = np.asarray(w_proj, dtype=np.float32)
    b_proj = np.asarray(b_proj, dtype=np.float32)

    xT = np.ascontiguousarray(
        np.transpose(x, (0, 2, 1))).astype(BF)                   # [B, D, S]
    wqkT = np.ascontiguousarray(w_attn[:2 * D].T).astype(BF)     # [D, 2D]
    wvT = np.ascontiguousarray(w_attn[2 * D:].T).astype(BF)      # [D, D]
    wpT = np.ascontiguousarray(w_proj.T).astype(BF)              # [D, D]
    bqk = np.ascontiguousarray(b_attn[:2 * D])
    bv = b_attn[2 * D:]
    beff = (b_proj.astype(np.float64)
            + w_proj.astype(np.float64) @ bv.astype(np.float64)
            ).astype(BF)
    umask = np.triu(np.ones((P, P), dtype=np.float32)).astype(BF)  # f >= p

    in_maps = [
        dict(xT=xT[b], wqkT=wqkT, wvT=wvT, wpT=wpT, bqk=bqk, beff=beff,
             umask=umask)
        for b in range(B)
    ]
    res = bass_utils.run_bass_kernel_spmd(
        nc, in_maps, core_ids=list(range(NCORES)), trace=TRACE)
    if TRACE:
        _CACHE["exec_time_ns"] = res.exec_time_ns
        _CACHE["trace"] = res.instructions_and_trace
    return np.stack([res.results[b]["y"] for b in range(B)], axis=0)


# revision 9
# speedup vs baseline: 1.4202x; 1.1072x over previous
"""Causal self-attention on 8 TRN2 NeuronCores, batch-data-parallel (one batch
element per core).

Layout strategy (per core, S=1024, D=1024, H=16, hd=64):
  - Host pre-transposes x -> xT [D,S] and all weights -> [in_dim, out_dim],
    and pre-rounds all matmul operands to bf16 (PE runs bf16 at 1 col/cycle
    like fp32r, but at roughly half the power -> far less HAM clock-gate
    throttling; DVE gets 2x on 16-bit; DMA volume halves).
  - qk projection produces q,k transposed ([e,s]) per head-pair: lhsT = wqkT
    tiles, rhs = xT.  Head h lives at partitions 64*(h%2)..+64.
  - v natural [s,e]: lhsT = xT tiles, rhs = wvT tiles; stored interleaved with
    a ones column per head (65 cols/head) so the AV matmul's PSUM row 64 is
    the softmax denominator (rowsum of unnormalized attn).
  - scoresT [sk,sq] per head-pair via K=64 matmuls; exp on ACT (scale=1/8
    folded in, bf16 out); causal diag masked by multiplicative
    upper-triangular mask; fully-masked tiles never computed.
  - AV: outT'[hd+1, sq] accumulated m-major in 512-wide chunks; normalization
    via approx-reciprocal of the den row + PE rank-1 broadcast + DVE multiply.
  - proj: y[s,e] with lhsT = outT tiles, rhs = resident wpT (prefetched during
    attention so the tail never waits on DMA) + rank-1 bias term
    (beff = b_proj + W_proj @ b_v; b_v folds exactly through softmax rowsum).
  - QKV matmul quanta are interleaved into the attention pair loop so the PE
    stream stays dense while ACT runs exp (keeps the HAM clock gate at 8/8).
  - xT is split into 8 per-ko tiles so the first QKV matmuls only wait on the
    first 256KB of DMA, not the full 2MB.
"""

import numpy as np

B, S, D, H = 8, 1024, 1024, 16
HD = D // H          # 64
P = 128
NCORES = 8
KO = D // P          # 8 contraction tiles over d
MT = (2 * D) // P    # 16 m-tiles for q,k
ST = S // P          # 8 s-tiles
NPAIRS = H // 2      # 8 head pairs

_CACHE = {}
TRACE = False        # set by test harness to collect an NTFF profile


def _score_chunks(w):
    # split w into pieces <=512 (PSUM bank limit); bf16 streams at full rate
    # at any width so no >=256 constraint
    table = {1024: [512, 512], 896: [512, 384], 768: [512, 256],
             640: [384, 256], 512: [512], 384: [384], 256: [256], 128: [128]}
    return table[w]


def _build():
    import concourse.tile as tile
    from concourse import bacc, mybir

    F32R = mybir.dt.float32r
    F32 = mybir.dt.float32
    BF16 = mybir.dt.bfloat16
    AF = mybir.ActivationFunctionType

    nc = bacc.Bacc("TRN2", target_bir_lowering=False, debug=False,
                   num_devices=NCORES)
    xT_d = nc.dram_tensor("xT", [D, S], BF16, kind="ExternalInput").ap()
    wqkT_d = nc.dram_tensor("wqkT", [D, 2 * D], BF16, kind="ExternalInput").ap()
    wvT_d = nc.dram_tensor("wvT", [D, D], BF16, kind="ExternalInput").ap()
    wpT_d = nc.dram_tensor("wpT", [D, D], BF16, kind="ExternalInput").ap()
    bqk_d = nc.dram_tensor("bqk", [2 * D], F32, kind="ExternalInput").ap()
    beff_d = nc.dram_tensor("beff", [D], BF16, kind="ExternalInput").ap()
    umask_d = nc.dram_tensor("umask", [P, P], BF16, kind="ExternalInput").ap()
    y_d = nc.dram_tensor("y", [S, D], F32, kind="ExternalOutput").ap()

    wqkT_v = wqkT_d.rearrange("(ko p) e -> p ko e", p=P)
    wvT_v = wvT_d.rearrange("(ko p) e -> p ko e", p=P)
    wpT_v = wpT_d.rearrange("(ko p) e -> p ko e", p=P)
    xT_v = xT_d.rearrange("(ko p) s -> p ko s", p=P)

    with tile.TileContext(nc) as tc:
        with (
            tc.tile_pool(name="bigio", bufs=1) as bigio,
            tc.tile_pool(name="qkp", bufs=3) as qkp,
            tc.tile_pool(name="vp", bufs=1) as vpool,
            tc.tile_pool(name="wqk", bufs=2) as wqkp,
            tc.tile_pool(name="wvp", bufs=1) as wvp,
            tc.tile_pool(name="attn", bufs=32) as attnp,
            tc.tile_pool(name="rt", bufs=2) as rtp,
            tc.tile_pool(name="rb", bufs=1) as rbp,
            tc.tile_pool(name="todd", bufs=1) as toddp,
            tc.tile_pool(name="ystg", bufs=2) as ystgp,
            tc.tile_pool(name="avsb", bufs=2) as avsbp,
            tc.tile_pool(name="cst", bufs=1) as cst,
            tc.tile_pool(name="psS", bufs=4, space="PSUM") as psS,
            tc.tile_pool(name="psAV", bufs=2, space="PSUM") as psAV,
        ):
            # ---------- constants ----------
            umask = cst.tile([P, P], BF16)
            nc.sync.dma_start(umask[:], umask_d)
            bqk_sb = cst.tile([P, MT], F32)
            nc.sync.dma_start(bqk_sb[:], bqk_d.rearrange("(m p) -> p m", p=P))
            beff_sb = cst.tile([1, D], BF16)
            nc.sync.dma_start(beff_sb[:], beff_d[None, :])
            onecol = cst.tile([P, 1], BF16)
            nc.vector.memset(onecol[:], 1.0)
            ones1x128 = cst.tile([1, P], BF16)
            nc.vector.tensor_copy(
                ones1x128[:], onecol[0:1, :].broadcast_to([1, P]))
            of32 = cst.tile([65, 64], F32)
            nc.vector.memset(of32[64:65, :], 1.0)
            ones65r = cst.tile([65, 64], F32R)
            nc.vector.tensor_copy(ones65r[64:65, :], of32[64:65, :])

            # ---------- big SBUF residents ----------
            # xT DMA'd in 4 column-chunks per ko: a single DMA queue moves
            # only ~20 GB/s, so parallel chunks cut the first-matmul wait
            xT = [bigio.tile([P, S], BF16, tag=f"xT{ko}", name=f"xT{ko}")
                  for ko in range(KO)]
            for ko in range(KO):
                for c in range(4):
                    nc.sync.dma_start(xT[ko][:, c * 256:(c + 1) * 256],
                                      xT_v[:, ko, c * 256:(c + 1) * 256])
            outT = bigio.tile([P, KO, S], BF16, tag="outT")
            wp_sb = bigio.tile([P, KO, D], BF16, tag="wp")
            v_sb = vpool.tile([P, ST, H * (HD + 1)], BF16)
            v_hview = v_sb[:].rearrange("p st (h c) -> p st h c", c=HD + 1)
            nc.vector.tensor_copy(
                v_hview[:, :, :, HD:HD + 1],
                onecol[:, None, None, :].broadcast_to([P, ST, H, 1]))

            qk_tiles = {}    # j -> [128, 2, S] tile (0=q, 1=k)

            # ---------- QKV work quanta (emitted interleaved) ----------
            def qk_quanta(j):
                # 4 closures; each computes one (part, nn) psum group
                t = qkp.tile([P, 2, S], BF16, tag="qkt", name=f"qk{j}")
                qk_tiles[j] = t
                wts = {}

                def quantum(part, nn):    # part 0=q (m-tile j), 1=k (8+j)
                    def go():
                        m = j if part == 0 else NPAIRS + j
                        if part not in wts:
                            wt = wqkp.tile([P, KO, P], BF16, tag="wqk", name=f"wqk{m}")
                            nc.sync.dma_start(
                                wt[:], wqkT_v[:, :, m * P:(m + 1) * P])
                            wts[part] = wt
                        wt = wts[part]
                        ps = psS.tile([P, 512], F32, tag="ps", name=f"qkps{m}")
                        for ko in range(KO):
                            nc.tensor.matmul(
                                ps[:], wt[:, ko, :],
                                xT[ko][:, nn * 512:(nn + 1) * 512],
                                start=(ko == 0), stop=(ko == KO - 1))
                        nc.vector.tensor_scalar_add(
                            t[:, part, nn * 512:(nn + 1) * 512], ps[:],
                            bqk_sb[:, m:m + 1])
                    return go
                return [quantum(0, 0), quantum(0, 1),
                        quantum(1, 0), quantum(1, 1)]

            # wv halves are made SBUF-resident ahead of use (8 parallel chunk
            # DMAs each) so v quanta never stall the PE on weight DMA
            wv_half = {}

            def load_wv(nE):
                t = wvp.tile([P, KO, 512], BF16, tag=f"wvh{nE}",
                             name=f"wvh{nE}")
                wv_half[nE] = t
                for ko in range(KO):
                    nc.sync.dma_start(t[:, ko, :],
                                      wvT_v[:, ko, nE * 512:(nE + 1) * 512])

            def v_quanta(nE):
                # v half nE: e_v cols 512*nE.. (heads 8nE..8nE+7), 4 quanta
                # of 2 s-tiles reading the resident weight half
                def quantum(g0):
                    def go():
                        sts = [g0, g0 + 1]
                        pss = {}
                        for st in sts:
                            pss[st] = psS.tile([P, 512], F32, tag="ps",
                                               name=f"vps{nE}_{st}")
                        wv_t = wv_half[nE]
                        for ko in range(KO):
                            for st in sts:
                                nc.tensor.matmul(
                                    pss[st][:],
                                    xT[ko][:, st * P:(st + 1) * P],
                                    wv_t[:, ko, :], start=(ko == 0),
                                    stop=(ko == KO - 1))
                        for st in sts:
                            nc.vector.tensor_copy(
                                v_hview[:, st, 8 * nE:8 * (nE + 1), 0:HD],
                                pss[st][:].rearrange("p (h c) -> p h c", c=HD))
                    return go
                return [quantum(g) for g in (0, 2, 4, 6)]

            # ---------- attention ----------
            pend = {}

            def scores_exp(j, m):
                qk_t = qk_tiles[j]
                w = S - m * P
                for hb, base in ((0, 0), (1, 64)):   # head 2j+hb
                    at = attnp.tile([P, S], BF16, tag="at",
                                    name=f"at{j}_{hb}_{m}")
                    pend[(j, hb, m)] = at
                    off = m * P
                    for cw in _score_chunks(w):
                        ps = psS.tile([P, 512], F32, tag="ps",
                                      name=f"sps{j}_{hb}_{m}")
                        nc.tensor.matmul(
                            ps[:, 0:cw],
                            qk_t[base:base + 64, 1, m * P:(m + 1) * P],
                            qk_t[base:base + 64, 0, off:off + cw],
                            start=True, stop=True)
                        nc.scalar.activation(
                            at[:, off:off + cw], ps[:, 0:cw], AF.Exp,
                            scale=0.125)
                        off += cw
                    nc.vector.tensor_mul(
                        at[:, m * P:(m + 1) * P], at[:, m * P:(m + 1) * P],
                        umask[:])

            def av_m(j, m):
                # exact chunking: chunk n starts at max(n*512, m*128) since
                # at[:, c] for c < m*128 is causally zero and never computed
                st8 = pend[f"ps{j}"]
                for hb in (0, 1):
                    h = 2 * j + hb
                    at = pend[(j, hb, m)]
                    for n in range(2):
                        c0 = max(n * 512, m * P)
                        c1 = (n + 1) * 512
                        if c0 >= c1:
                            continue
                        nc.tensor.matmul(
                            st8[hb][:, c0:c1],
                            v_sb[:, m, h * (HD + 1):(h + 1) * (HD + 1)],
                            at[:, c0:c1],
                            start=(m == 0), stop=(m == 4 * n + 3))

            def evict_recip(j):
                # move the [65, S] AV accumulators out of PSUM so the next
                # pair's AV matmuls get the PSUM slots immediately, then take
                # the reciprocal of the den row (approx_fast: ~4e-6 rel, 5x
                # faster than exact; den >= exp(0) > 0 so no edge cases)
                avcs, recs = [], []
                for hb in (0, 1):
                    avc = avsbp.tile([65, S], F32R, tag="avc",
                                     name=f"avc{j}_{hb}")
                    nc.vector.tensor_copy(avc[:], pend[f"ps{j}"][hb][:])
                    avcs.append(avc)
                    rt = rtp.tile([65, S], F32R, tag="rt")
                    rt32 = rtp.tile([65, S], F32, tag="rt32", bufs=1)
                    # custom-DVE op misbehaves on single-partition APs on HW:
                    # run it over all 65 rows (lanes are parallel) and consume
                    # only the den row (64); other lanes are never read
                    nc.vector.reciprocal_approx_fast(
                        rt32[:], avc[:].bitcast(F32))
                    nc.vector.tensor_copy(rt[64:65, :], rt32[64:65, :])
                    recs.append(rt)
                pend[f"avc{j}"] = avcs
                pend[f"rec{j}"] = recs
                del pend[f"ps{j}"]

            def rb_norm(j):
                for hb in (0, 1):
                    rt = pend[f"rec{j}"][hb]
                    rb_t = rbp.tile([64, S], F32R, tag="rb")
                    for c in range(2):
                        rps = psS.tile([P, 512], F32, tag="ps",
                                       name=f"rbps{j}_{hb}_{c}")
                        nc.tensor.matmul(
                            rps[0:64, :], ones65r[64:65, :],
                            rt[64:65, c * 512:(c + 1) * 512],
                            start=True, stop=True)
                        nc.vector.tensor_copy(
                            rb_t[:, c * 512:(c + 1) * 512], rps[0:64, :])
                    avc = pend[f"avc{j}"][hb]
                    if hb == 0:
                        nc.vector.tensor_mul(
                            outT[0:64, j, :], avc[0:64, :], rb_t[:])
                    else:
                        # DVE lanes cannot shift partitions: multiply to an
                        # SBUF tmp, then DMA-shift rows 0..63 -> 64..127
                        tmp = toddp.tile([64, S], BF16, tag="todd")
                        nc.vector.tensor_mul(tmp[:], avc[0:64, :], rb_t[:])
                        nc.sync.dma_start(outT[64:128, j, :], tmp[:])
                del pend[f"avc{j}"], pend[f"rec{j}"]

            def load_wp(half):
                # projection weights prefetched mid-attention in parallel
                # 512-col chunk DMAs; resident wp kills the tail's DMA waits
                for ko in range(4 * half, 4 * half + 4):
                    for c in range(2):
                        nc.sync.dma_start(
                            wp_sb[:, ko, c * 512:(c + 1) * 512],
                            wpT_v[:, ko, c * 512:(c + 1) * 512])

            # ---------- interleaved emission ----------
            # software pipeline: scores/exp for pair j+1 are emitted during
            # pair j's AV so the PE never waits on ACT exp drain, and the
            # last pairs still have dense PE work.
            # prologue: qk pairs 0-2, wv half 0, v half 0 + scores pair 0
            for q in qk_quanta(0):
                q()
            for q in qk_quanta(1):
                q()
            load_wv(0)
            vq0 = v_quanta(0)
            for g in range(4):
                vq0[g]()
                scores_exp(0, 2 * g)
                scores_exp(0, 2 * g + 1)
            for q in qk_quanta(2):
                q()

            vwork = []
            for j in range(NPAIRS):
                work = []
                if j + 3 < NPAIRS:
                    work.extend(qk_quanta(j + 3))
                if j == 0:
                    work.append(lambda: load_wv(1))
                if j == 1:
                    vwork = v_quanta(1)
                if j in (1, 2):
                    work.append(vwork.pop(0))
                    work.append(vwork.pop(0))
                if j in (2, 3):
                    work.append(lambda h=j - 2: load_wp(h))
                for m in range(ST):
                    if j + 1 < NPAIRS:
                        scores_exp(j + 1, m)
                    if m == 4 and j > 0:
                        rb_norm(j - 1)
                    if m == 0:
                        pend[f"ps{j}"] = [
                            psAV.tile([65, S], F32, tag="av",
                                      name=f"av{j}_{hb}") for hb in range(2)]
                    av_m(j, m)
                    if m % 2 == 1 and work:
                        work.pop(0)()
                        if len(work) > (ST - 1 - m) // 2:
                            work.pop(0)()
                while work:
                    work.pop(0)()
                evict_recip(j)
            rb_norm(NPAIRS - 1)

            # ---------- output projection ----------
            for g0 in range(0, ST, 3):
                sts = list(range(g0, min(g0 + 3, ST)))
                for nE in range(2):
                    pss = {st: psS.tile([P, 512], F32, tag="ps",
                                        name=f"yps{st}") for st in sts}
                    for ko in range(KO):
                        for st in sts:
                            nc.tensor.matmul(
                                pss[st][:],
                                outT[:, ko, st * P:(st + 1) * P],
                                wp_sb[:, ko, nE * 512:(nE + 1) * 512],
                                start=(ko == 0), stop=False)
                    for st in sts:
                        nc.tensor.matmul(
                            pss[st][:], ones1x128[:],
                            beff_sb[:, nE * 512:(nE + 1) * 512],
                            start=False, stop=True)
                        ystg = ystgp.tile([P, 512], F32, tag="ystg",
                                          name=f"ystg{st}")
                        nc.vector.tensor_copy(ystg[:], pss[st][:])
                        nc.sync.dma_start(
                            y_d[st * P:(st + 1) * P, nE * 512:(nE + 1) * 512],
                            ystg[:])

    nc.compile()
    return nc


def kernel(x, w_attn, b_attn, w_proj, b_proj):
    import concourse.bass_utils as bass_utils
    import ml_dtypes

    if "nc" not in _CACHE:
        _CACHE["nc"] = _build()
    nc = _CACHE["nc"]

    BF = ml_dtypes.bfloat16
    x = np.asarray(x, dtype=np.float32)
    w_attn = np.asarray(w_attn, dtype=np.float32)
    b_attn = np.asarray(b_attn, dtype=np.float32)
    w_proj = np.asarray(w_proj, dtype=np.float32)
    b_proj = np.asarray(b_proj, dtype=np.float32)

    xT = np.ascontiguousarray(
        np.transpose(x, (0, 2, 1))).astype(BF)                   # [B, D, S]
    wqkT = np.ascontiguousarray(w_attn[:2 * D].T).astype(BF)     # [D, 2D]
    wvT = np.ascontiguousarray(w_attn[2 * D:].T).astype(BF)      # [D, D]
    wpT = np.ascontiguousarray(w_proj.T).astype(BF)              # [D, D]
    bqk = np.ascontiguousarray(b_attn[:2 * D])
    bv = b_attn[2 * D:]
    beff = (b_proj.astype(np.float64)
            + w_proj.astype(np.float64) @ bv.astype(np.float64)
            ).astype(BF)
    umask = np.triu(np.ones((P, P), dtype=np.float32)).astype(BF)  # f >= p

    in_maps = [
        dict(xT=xT[b], wqkT=wqkT, wvT=wvT, wpT=wpT, bqk=bqk, beff=beff,
             umask=umask)
        for b in range(B)
    ]
    res = bass_utils.run_bass_kernel_spmd(
        nc, in_maps, core_ids=list(range(NCORES)), trace=TRACE)
    if TRACE:
        _CACHE["exec_time_ns"] = res.exec_time_ns
        _CACHE["trace"] = res.instructions_and_trace
    return np.stack([res.results[b]["y"] for b in range(B)], axis=0)


# revision 26
# speedup vs baseline: 1.4472x; 1.0191x over previous
"""Causal self-attention on 8 TRN2 NeuronCores, batch-data-parallel (one batch
element per core).

Layout strategy (per core, S=1024, D=1024, H=16, hd=64):
  - Host pre-transposes x -> xT [D,S], weights -> [in_dim, out_dim], and
    pre-rounds all matmul operands to bf16 (PE runs bf16 at 1 col/cycle like
    fp32r but at about half the power -> far less HAM clock-gate throttling;
    DVE gets 2x on 16-bit; DMA volume halves).  wqk is additionally
    host-relayouted to [p, m, ko, c] so each m-tile's weight DMA reads 2KB
    contiguous lines (the [D,2D] layout gave 256B lines at ~25% DMA
    efficiency).  All bulk DMAs use 2KB lines and are partition-split in two
    so two queues (~21 GB/s each) carry every tile.
  - qk projection produces q,k transposed ([e,s]) per head-pair: lhsT = wqkT
    tiles, rhs = xT.  Head h lives at partitions 64*(h%2)..+64.
  - v natural [s,e]: lhsT = xT tiles, rhs = resident wvT; stored interleaved
    with a ones column per head (65 cols/head) so the AV matmul's PSUM row
    64 is the softmax denominator (rowsum of unnormalized attn).  Odd heads'
    normalized output is DMA-shifted to outT rows 64..127 (partition-split
    across two queues; lane engines cannot cross partitions).
  - scoresT [sk,sq] per head-pair via K=64 matmuls; exp on ACT (scale=1/8
    folded in, bf16 out); the causal diagonal mask is a multiplicative
    [128,128] tensor_mul on the otherwise-idle GPSIMD engine (keeps PE and
    DVE free).  Fully-masked tiles are never computed.
  - AV: chunks start exactly at the causal boundary (at[:, c<128m] is never
    written); accumulated m-major into [128,S] PSUM tiles; normalization via
    approx-reciprocal of the den row (bitcast to f32r in place) + PE rank-1
    broadcast + DVE multiply.
  - proj: y[s,e] with lhsT = outT tiles, rhs = resident wpT (prefetched
    mid-attention) + rank-1 bias term (beff = b_proj + W_proj @ b_v).
  - software pipeline: scores/exp for pair j+1 are emitted during pair j's
    AV so the PE stream stays dense while ACT drains exp; QKV matmul quanta
    fill the remaining PE slack.  Pair 7 runs its AV head-major so the
    evict/recip/normalize chain of head 14 overlaps head 15's AV, and the
    first projection group's ko 0..6 matmuls are emitted before pair 7's
    normalize so the PE never waits on the tail chain.
"""

import numpy as np

B, S, D, H = 8, 1024, 1024, 16
HD = D // H          # 64
P = 128
NCORES = 8
KO = D // P          # 8 contraction tiles over d
MT = (2 * D) // P    # 16 m-tiles for q,k
ST = S // P          # 8 s-tiles
NPAIRS = H // 2      # 8 head pairs

_CACHE = {}
TRACE = False        # set by test harness to collect an NTFF profile


def _score_chunks(w):
    # split w into pieces <=512 (PSUM bank limit); bf16 streams at full rate
    # at any width so no >=256 constraint
    table = {1024: [512, 512], 896: [512, 384], 768: [512, 256],
             640: [384, 256], 512: [512], 384: [384], 256: [256], 128: [128]}
    return table[w]


def _build():
    import concourse.tile as tile
    from concourse import bacc, mybir

    F32R = mybir.dt.float32r
    F32 = mybir.dt.float32
    BF16 = mybir.dt.bfloat16
    AF = mybir.ActivationFunctionType

    nc = bacc.Bacc("TRN2", target_bir_lowering=False, debug=False,
                   num_devices=NCORES)
    xT_d = nc.dram_tensor("xT", [D, S], BF16, kind="ExternalInput").ap()
    wqk_d = nc.dram_tensor("wqk2", [P, MT, KO, P], BF16,
                           kind="ExternalInput").ap()
    wvT_d = nc.dram_tensor("wvT", [D, D], BF16, kind="ExternalInput").ap()
    wpT_d = nc.dram_tensor("wpT", [D, D], BF16, kind="ExternalInput").ap()
    bqk_d = nc.dram_tensor("bqk", [2 * D], F32, kind="ExternalInput").ap()
    beff_d = nc.dram_tensor("beff", [D], BF16, kind="ExternalInput").ap()
    umask_d = nc.dram_tensor("umask", [P, P], BF16, kind="ExternalInput").ap()
    y_d = nc.dram_tensor("y", [S, D], F32, kind="ExternalOutput").ap()

    wvT_v = wvT_d.rearrange("(ko p) e -> p ko e", p=P)
    wpT_v = wpT_d.rearrange("(ko p) e -> p ko e", p=P)
    xT_v = xT_d.rearrange("(ko p) s -> p ko s", p=P)

    with tile.TileContext(nc) as tc:
        with (
            tc.tile_pool(name="bigio", bufs=1) as bigio,
            tc.tile_pool(name="qkp", bufs=3) as qkp,
            tc.tile_pool(name="vp", bufs=1) as vpool,
            tc.tile_pool(name="wqk", bufs=4) as wqkp,
            tc.tile_pool(name="attn", bufs=20) as attnp,
            tc.tile_pool(name="rt", bufs=2) as rtp,
            tc.tile_pool(name="rb", bufs=2) as rbp,
            tc.tile_pool(name="todd", bufs=2) as toddp,
            tc.tile_pool(name="ystg", bufs=2) as ystgp,
            tc.tile_pool(name="avsb", bufs=2) as avsbp,
            tc.tile_pool(name="cst", bufs=1) as cst,
            tc.tile_pool(name="psS", bufs=4, space="PSUM") as psS,
            tc.tile_pool(name="psAV", bufs=2, space="PSUM") as psAV,
        ):
            def dma2(dst, src):
                # partition-split DMA: two queues per tile, 2KB lines
                nc.sync.dma_start(dst[0:64], src[0:64])
                nc.sync.dma_start(dst[64:128], src[64:128])

            # ---------- constants ----------
            umask = cst.tile([P, P], BF16)
            nc.sync.dma_start(umask[:], umask_d)
            bqk_sb = cst.tile([P, MT], F32)
            nc.sync.dma_start(bqk_sb[:], bqk_d.rearrange("(m p) -> p m", p=P))
            beff_sb = cst.tile([1, D], BF16)
            nc.sync.dma_start(beff_sb[:], beff_d[None, :])
            onecol = cst.tile([P, 1], BF16)
            nc.vector.memset(onecol[:], 1.0)
            ones1x128 = cst.tile([1, P], BF16)
            nc.vector.tensor_copy(
                ones1x128[:], onecol[0:1, :].broadcast_to([1, P]))
            of32 = cst.tile([65, 64], F32)
            nc.vector.memset(of32[64:65, :], 1.0)
            ones65r = cst.tile([65, 64], F32R)
            nc.vector.tensor_copy(ones65r[64:65, :], of32[64:65, :])

            # ---------- big SBUF residents ----------
            # wq/wk for the first pairs, then xT, then the rest
            xT = [bigio.tile([P, S], BF16, tag=f"xT{ko}", name=f"xT{ko}")
                  for ko in range(KO)]
            wqk_tiles = {}

            def load_wqk(m):
                wt = wqkp.tile([P, KO, P], BF16, tag="wqk", name=f"wqk{m}")
                nc.sync.dma_start(wt[:], wqk_d[:, m, :, :])
                wqk_tiles[m] = wt

            load_wqk(0)
            load_wqk(NPAIRS)
            for ko in range(KO):
                dma2(xT[ko], xT_v[:, ko, :])
            outT = bigio.tile([P, KO, S], BF16, tag="outT")
            wp_sb = bigio.tile([P, KO, D], BF16, tag="wp")
            wv_sb = bigio.tile([P, KO, D], BF16, tag="wv")
            for m in (1, NPAIRS + 1, 2, NPAIRS + 2):
                load_wqk(m)
            for ko in range(KO):
                dma2(wv_sb[:, ko, :], wvT_v[:, ko, :])

            v_sb = vpool.tile([P, ST, H * (HD + 1)], BF16)
            v_hview = v_sb[:].rearrange("p st (h c) -> p st h c", c=HD + 1)
            nc.vector.tensor_copy(
                v_hview[:, :, :, HD:HD + 1],
                onecol[:, None, None, :].broadcast_to([P, ST, H, 1]))

            qk_tiles = {}    # j -> [128, 2, S] tile (0=q, 1=k)

            # ---------- QKV work quanta (emitted interleaved) ----------
            def qk_quanta(j):
                # 4 closures; each computes one (part, nn) psum group
                t = qkp.tile([P, 2, S], BF16, tag="qkt", name=f"qk{j}")
                qk_tiles[j] = t

                def quantum(part, nn):    # part 0=q (m-tile j), 1=k (8+j)
                    def go():
                        m = j if part == 0 else NPAIRS + j
                        wt = wqk_tiles[m]
                        ps = psS.tile([P, 512], F32, tag="ps", name=f"qkps{m}")
                        for ko in range(KO):
                            nc.tensor.matmul(
                                ps[:], wt[:, ko, :],
                                xT[ko][:, nn * 512:(nn + 1) * 512],
                                start=(ko == 0), stop=(ko == KO - 1))
                        nc.vector.tensor_scalar_add(
                            t[:, part, nn * 512:(nn + 1) * 512], ps[:],
                            bqk_sb[:, m:m + 1])
                    return go
                return [quantum(0, 0), quantum(0, 1),
                        quantum(1, 0), quantum(1, 1)]

            def v_quanta(nE):
                # v half nE: e_v cols 512*nE.. (heads 8nE..8nE+7), 4 quanta
                # of 2 s-tiles reading the resident weight
                def quantum(g0):
                    def go():
                        sts = [g0, g0 + 1]
                        pss = {}
                        for st in sts:
                            pss[st] = psS.tile([P, 512], F32, tag="ps",
                                               name=f"vps{nE}_{st}")
                        for ko in range(KO):
                            for st in sts:
                                nc.tensor.matmul(
                                    pss[st][:],
                                    xT[ko][:, st * P:(st + 1) * P],
                                    wv_sb[:, ko, nE * 512:(nE + 1) * 512],
                                    start=(ko == 0), stop=(ko == KO - 1))
                        for st in sts:
                            nc.vector.tensor_copy(
                                v_hview[:, st, 8 * nE:8 * (nE + 1), 0:HD],
                                pss[st][:].rearrange("p (h c) -> p h c",
                                                     c=HD))
                    return go
                return [quantum(g) for g in (0, 2, 4, 6)]

            # ---------- attention ----------
            pend = {}

            def scores_exp(j, m):
                qk_t = qk_tiles[j]
                w = S - m * P
                for hb, base in ((0, 0), (1, 64)):   # head 2j+hb
                    at = attnp.tile([P, S], BF16, tag="at",
                                    name=f"at{j}_{hb}_{m}")
                    pend[(j, hb, m)] = at
                    off = m * P
                    for cw in _score_chunks(w):
                        ps = psS.tile([P, 512], F32, tag="ps",
                                      name=f"sps{j}_{hb}_{m}")
                        nc.tensor.matmul(
                            ps[:, 0:cw],
                            qk_t[base:base + 64, 1, m * P:(m + 1) * P],
                            qk_t[base:base + 64, 0, off:off + cw],
                            start=True, stop=True)
                        nc.scalar.activation(
                            at[:, off:off + cw], ps[:, 0:cw], AF.Exp,
                            scale=0.125)
                        off += cw
                    # causal diagonal mask on the (otherwise idle) GPSIMD
                    # engine: SBUF->SBUF multiply, keeps both PE and DVE free
                    nc.gpsimd.tensor_mul(
                        at[:, m * P:(m + 1) * P], at[:, m * P:(m + 1) * P],
                        umask[:])

            def av_m(j, m, hbs=(0, 1)):
                # exact chunking: chunk n starts at max(n*512, m*128) since
                # at[:, c] for c < m*128 is causally zero and never computed
                st8 = pend[f"ps{j}"]
                for hb in hbs:
                    h = 2 * j + hb
                    at = pend[(j, hb, m)]
                    out = st8[hb][0:65]
                    for n in range(2):
                        c0 = max(n * 512, m * P)
                        c1 = (n + 1) * 512
                        if c0 >= c1:
                            continue
                        nc.tensor.matmul(
                            out[:, c0:c1],
                            v_sb[:, m, h * (HD + 1):(h + 1) * (HD + 1)],
                            at[:, c0:c1],
                            start=(m == 0), stop=(m == 4 * n + 3))

            def evict_recip(j, hbs=(0, 1)):
                # move the AV accumulators out of PSUM so the next pair's AV
                # matmuls get the PSUM slots immediately (the tensor_copy
                # rounds to f32r, which legalizes row 64 as a f32r matmul
                # operand in rb_norm)
                for hb in hbs:
                    avc = avsbp.tile([P, S], F32R, tag="avc",
                                     name=f"avc{j}_{hb}")
                    nc.vector.tensor_copy(avc[0:65],
                                          pend[f"ps{j}"][hb][0:65])
                    pend[f"avc{j}_{hb}"] = avc
                if hbs == (0, 1) or hbs == (1,):
                    del pend[f"ps{j}"]

            def rb_norm(j, hbs=(0, 1)):
                # broadcast the RAW den row with a rank-1 f32r matmul, then
                # reciprocal AFTER the broadcast on the [64,S] tile
                # (approx_fast: ~4e-6 rel; den >= exp(0) > 0 so no edge
                # cases; multi-partition so the custom-DVE op is safe, and
                # only DVE consumes it so no f32r-rounding constraint)
                for hb in hbs:
                    avc = pend[f"avc{j}_{hb}"]
                    rb32 = rbp.tile([P, S], F32, tag="rb",
                                    name=f"rb{j}_{hb}")
                    for c in range(2):
                        rps = psS.tile([P, 512], F32, tag="ps",
                                       name=f"rbps{j}_{hb}_{c}")
                        nc.tensor.matmul(
                            rps[0:64, :], ones65r[64:65, :],
                            avc[64:65, c * 512:(c + 1) * 512],
                            start=True, stop=True)
                        nc.vector.tensor_copy(
                            rb32[0:64, c * 512:(c + 1) * 512], rps[0:64, :])
                    rbr = rtp.tile([P, S], F32, tag="rt", name=f"rbr{j}_{hb}")
                    nc.vector.reciprocal_approx_fast(
                        rbr[0:64, :], rb32[0:64, :])
                    if hb == 0:
                        nc.vector.tensor_mul(
                            outT[0:64, j, :], avc[0:64, :], rbr[0:64, :])
                    else:
                        # DVE lanes cannot shift partitions: multiply to an
                        # SBUF tmp, then DMA-shift rows 0..63 -> 64..127
                        # (partition-split across two queues)
                        tmp = toddp.tile([64, S], BF16, tag="todd",
                                         name=f"todd{j}")
                        nc.vector.tensor_mul(tmp[:], avc[0:64, :],
                                             rbr[0:64, :])
                        nc.sync.dma_start(outT[64:96, j, :], tmp[0:32])
                        nc.sync.dma_start(outT[96:128, j, :], tmp[32:64])
                    del pend[f"avc{j}_{hb}"]

            def load_wp(half):
                # projection weights prefetched mid-attention; resident wp
                # kills the tail's DMA waits
                for ko in range(4 * half, 4 * half + 4):
                    dma2(wp_sb[:, ko, :], wpT_v[:, ko, :])

            # ---------- output projection ----------
            def proj_group(g, split_tail=None):
                # group g: 2 s-tiles x one nE half; ko ascending so the
                # pair-7-dependent ko=7 comes last
                nE, sts = g % 2, [2 * (g // 2), 2 * (g // 2) + 1]
                pss = {st: psS.tile([P, 512], F32, tag="ps",
                                    name=f"yps{st}_{nE}") for st in sts}
                for ko in range(7):
                    for st in sts:
                        nc.tensor.matmul(
                            pss[st][:], outT[:, ko, st * P:(st + 1) * P],
                            wp_sb[:, ko, nE * 512:(nE + 1) * 512],
                            start=(ko == 0), stop=False)
                if split_tail is not None:
                    split_tail()
                for st in sts:
                    nc.tensor.matmul(
                        pss[st][:], outT[:, 7, st * P:(st + 1) * P],
                        wp_sb[:, 7, nE * 512:(nE + 1) * 512],
                        start=False, stop=False)
                    nc.tensor.matmul(
                        pss[st][:], ones1x128[:],
                        beff_sb[:, nE * 512:(nE + 1) * 512],
                        start=False, stop=True)
                    ystg = ystgp.tile([P, 512], F32, tag="ystg",
                                      name=f"ystg{st}_{nE}")
                    nc.vector.tensor_copy(ystg[:], pss[st][:])
                    dma2(y_d[st * P:(st + 1) * P, nE * 512:(nE + 1) * 512],
                         ystg)

            # ---------- interleaved emission ----------
            # software pipeline: scores/exp for pair j+1 are emitted during
            # pair j's AV; QKV quanta fill the remaining PE slack
            for q in qk_quanta(0):
                q()
            for q in qk_quanta(1):
                q()
            vq0 = v_quanta(0)
            for g in range(4):
                vq0[g]()
                scores_exp(0, 2 * g)
                scores_exp(0, 2 * g + 1)
            for q in qk_quanta(2):
                q()

            vwork = []
            for j in range(NPAIRS - 1):
                work = []
                if j + 3 < NPAIRS:
                    work.append(lambda m=j + 3: load_wqk(m))
                    work.append(lambda m=NPAIRS + j + 3: load_wqk(m))
                    work.extend(qk_quanta(j + 3))
                if j == 1:
                    vwork = v_quanta(1)
                if j in (1, 2):
                    work.append(vwork.pop(0))
                    work.append(vwork.pop(0))
                if j in (2, 3):
                    work.append(lambda h=j - 2: load_wp(h))
                for m in range(ST):
                    scores_exp(j + 1, m)
                    if m == 4 and j > 0:
                        rb_norm(j - 1)
                    if m == 0:
                        pend[f"ps{j}"] = [
                            psAV.tile([P, S], F32, tag="av",
                                      name=f"av{j}_{hb}") for hb in range(2)]
                    av_m(j, m)
                    if m % 2 == 1 and work:
                        work.pop(0)()
                        if len(work) > (ST - 1 - m) // 2:
                            work.pop(0)()
                while work:
                    work.pop(0)()
                evict_recip(j)

            # pair 7: head-major AV so head 14's evict/recip/normalize chain
            # overlaps head 15's AV matmuls
            j = NPAIRS - 1
            pend[f"ps{j}"] = [psAV.tile([P, S], F32, tag="av",
                                        name=f"av{j}_{hb}") for hb in range(2)]
            for m in range(ST):
                av_m(j, m, hbs=(0,))
                if m == 3:
                    rb_norm(j - 1)
            evict_recip(j, hbs=(0,))
            for m in range(ST):
                av_m(j, m, hbs=(1,))
                if m == 3:
                    rb_norm(j, hbs=(0,))
            evict_recip(j, hbs=(1,))
            # first proj group's ko 0..6 run while pair 7 head 15 normalizes
            proj_group(0, split_tail=lambda: rb_norm(j, hbs=(1,)))
            for g in range(1, 2 * (ST // 2)):
                proj_group(g)

    nc.compile()
    return nc


def kernel(x, w_attn, b_attn, w_proj, b_proj):
    import concourse.bass_utils as bass_utils
    import ml_dtypes

    if "nc" not in _CACHE:
        _CACHE["nc"] = _build()
    nc = _CACHE["nc"]

    BF = ml_dtypes.bfloat16
    x = np.asarray(x, dtype=np.float32)
    w_attn = np.asarray(w_attn, dtype=np.float32)
    b_attn = np.asarray(b_attn, dtype=np.float32)
    w_proj = np.asarray(w_proj, dtype=np.float32)
    b_proj = np.asarray(b_proj, dtype=np.float32)

    xT = np.ascontiguousarray(
        np.transpose(x, (0, 2, 1))).astype(BF)                   # [B, D, S]
    wqkT = w_attn[:2 * D].T                                      # [D, 2D]
    # [p, m, ko, c] relayout so each m-tile weight DMA reads 2KB lines
    wqk2 = np.ascontiguousarray(
        wqkT.reshape(KO, P, MT, P).transpose(1, 2, 0, 3)).astype(BF)
    wvT = np.ascontiguousarray(w_attn[2 * D:].T).astype(BF)      # [D, D]
    wpT = np.ascontiguousarray(w_proj.T).astype(BF)              # [D, D]
    bqk = np.ascontiguousarray(b_attn[:2 * D])
    bv = b_attn[2 * D:]
    beff = (b_proj.astype(np.float64)
            + w_proj.astype(np.float64) @ bv.astype(np.float64)
            ).astype(BF)
    umask = np.triu(np.ones((P, P), dtype=np.float32)).astype(BF)  # f >= p

    in_maps = [
        dict(xT=xT[b], wqk2=wqk2, wvT=wvT, wpT=wpT, bqk=bqk, beff=beff,
             umask=umask)
        for b in range(B)
    ]
    res = bass_utils.run_bass_kernel_spmd(
        nc, in_maps, core_ids=list(range(NCORES)), trace=TRACE)
    if TRACE:
        _CACHE["exec_time_ns"] = res.exec_time_ns
        _CACHE["trace"] = res.instructions_and_trace
    return np.stack([res.results[b]["y"] for b in range(B)], axis=0)


# revision 33
# speedup vs baseline: 1.4565x; 1.0064x over previous
"""Causal self-attention on 8 TRN2 NeuronCores, batch-data-parallel (one batch
element per core).

Layout strategy (per core, S=1024, D=1024, H=16, hd=64):
  - Host pre-transposes x -> xT [D,S], weights -> [in_dim, out_dim], and
    pre-rounds all matmul operands to bf16 (PE runs bf16 at 1 col/cycle like
    fp32r but at about half the power -> far less HAM clock-gate throttling;
    DVE gets 2x on 16-bit; DMA volume halves).  wqk is additionally
    host-relayouted to [p, m, ko, c] so each m-tile's weight DMA reads 2KB
    contiguous lines (the [D,2D] layout gave 256B lines at ~25% DMA
    efficiency).  All bulk DMAs use 2KB lines and are partition-split in two
    so two queues (~21 GB/s each) carry every tile.
  - qk projection produces q,k transposed ([e,s]) per head-pair: lhsT = wqkT
    tiles, rhs = xT.  Head h lives at partitions 64*(h%2)..+64.
  - v natural [s,e]: lhsT = xT tiles, rhs = resident wvT; stored interleaved
    with a ones column per head (65 cols/head) so the AV matmul's PSUM row
    64 is the softmax denominator (rowsum of unnormalized attn).  Odd heads'
    normalized output is DMA-shifted to outT rows 64..127 (partition-split
    across two queues; lane engines cannot cross partitions).
  - scoresT [sk,sq] per head-pair via K=64 matmuls; exp on ACT (scale=1/8
    folded in, bf16 out); the causal diagonal mask is a multiplicative
    [128,128] tensor_mul on the otherwise-idle GPSIMD engine (keeps PE and
    DVE free).  Fully-masked tiles are never computed.
  - AV: chunks start exactly at the causal boundary (at[:, c<128m] is never
    written); accumulated m-major into [128,S] PSUM tiles; normalization via
    approx-reciprocal of the den row (bitcast to f32r in place) + PE rank-1
    broadcast + DVE multiply.
  - proj: y[s,e] with lhsT = outT tiles, rhs = resident wpT (prefetched
    mid-attention) + rank-1 bias term (beff = b_proj + W_proj @ b_v).
  - software pipeline: scores/exp for pair j+1 are emitted during pair j's
    AV so the PE stream stays dense while ACT drains exp; QKV matmul quanta
    fill the remaining PE slack.  Pair 7 runs its AV head-major so the
    evict/recip/normalize chain of head 14 overlaps head 15's AV, and the
    first projection group's ko 0..6 matmuls are emitted before pair 7's
    normalize so the PE never waits on the tail chain.
"""

import numpy as np

B, S, D, H = 8, 1024, 1024, 16
HD = D // H          # 64
P = 128
NCORES = 8
KO = D // P          # 8 contraction tiles over d
MT = (2 * D) // P    # 16 m-tiles for q,k
ST = S // P          # 8 s-tiles
NPAIRS = H // 2      # 8 head pairs

_CACHE = {}
TRACE = False        # set by test harness to collect an NTFF profile


def _score_chunks(w):
    # split w into pieces <=512 (PSUM bank limit); bf16 streams at full rate
    # at any width so no >=256 constraint
    table = {1024: [512, 512], 896: [512, 384], 768: [512, 256],
             640: [384, 256], 512: [512], 384: [384], 256: [256], 128: [128]}
    return table[w]


def _build():
    import concourse.tile as tile
    from concourse import bacc, mybir

    F32R = mybir.dt.float32r
    F32 = mybir.dt.float32
    BF16 = mybir.dt.bfloat16
    AF = mybir.ActivationFunctionType

    nc = bacc.Bacc("TRN2", target_bir_lowering=False, debug=False,
                   num_devices=NCORES)
    xT_d = nc.dram_tensor("xT", [D, S], BF16, kind="ExternalInput").ap()
    wqk_d = nc.dram_tensor("wqk2", [P, MT, KO, P], BF16,
                           kind="ExternalInput").ap()
    wvT_d = nc.dram_tensor("wvT", [D, D], BF16, kind="ExternalInput").ap()
    wpT_d = nc.dram_tensor("wpT", [D, D], BF16, kind="ExternalInput").ap()
    bqk_d = nc.dram_tensor("bqk", [2 * D], F32, kind="ExternalInput").ap()
    beff_d = nc.dram_tensor("beff", [D], BF16, kind="ExternalInput").ap()
    umask_d = nc.dram_tensor("umask", [P, P], BF16, kind="ExternalInput").ap()
    y_d = nc.dram_tensor("y", [S, D], F32, kind="ExternalOutput").ap()

    wvT_v = wvT_d.rearrange("(ko p) e -> p ko e", p=P)
    wpT_v = wpT_d.rearrange("(ko p) e -> p ko e", p=P)
    xT_v = xT_d.rearrange("(ko p) s -> p ko s", p=P)

    with tile.TileContext(nc) as tc:
        with (
            tc.tile_pool(name="bigio", bufs=1) as bigio,
            tc.tile_pool(name="qkp", bufs=3) as qkp,
            tc.tile_pool(name="vp", bufs=1) as vpool,
            tc.tile_pool(name="wqk", bufs=4) as wqkp,
            tc.tile_pool(name="attn", bufs=20) as attnp,
            tc.tile_pool(name="rt", bufs=2) as rtp,
            tc.tile_pool(name="rb", bufs=2) as rbp,
            tc.tile_pool(name="todd", bufs=2) as toddp,
            tc.tile_pool(name="ystg", bufs=2) as ystgp,
            tc.tile_pool(name="avsb", bufs=2) as avsbp,
            tc.tile_pool(name="cst", bufs=1) as cst,
            tc.tile_pool(name="psS", bufs=4, space="PSUM") as psS,
            tc.tile_pool(name="psAV", bufs=2, space="PSUM") as psAV,
        ):
            def dma2(dst, src):
                # partition-split DMA: two queues per tile, 2KB lines
                nc.sync.dma_start(dst[0:64], src[0:64])
                nc.sync.dma_start(dst[64:128], src[64:128])

            # first-needed tiles go first: the earliest DMA queues start
            # ~3us before the bulk rings, so the first matmul's operands
            # ride them
            wqk_tiles = {}

            def load_wqk(m):
                wt = wqkp.tile([P, KO, P], BF16, tag="wqk", name=f"wqk{m}")
                nc.sync.dma_start(wt[:], wqk_d[:, m, :, :])
                wqk_tiles[m] = wt

            xT = [bigio.tile([P, S], BF16, tag=f"xT{ko}", name=f"xT{ko}")
                  for ko in range(KO)]
            load_wqk(0)
            load_wqk(NPAIRS)
            dma2(xT[0], xT_v[:, 0, :])

            # ---------- constants ----------
            umask = cst.tile([P, P], BF16)
            nc.sync.dma_start(umask[:], umask_d)
            bqk_sb = cst.tile([P, MT], F32)
            nc.sync.dma_start(bqk_sb[:], bqk_d.rearrange("(m p) -> p m", p=P))
            beff_sb = cst.tile([1, D], BF16)
            nc.sync.dma_start(beff_sb[:], beff_d[None, :])
            onecol = cst.tile([P, 1], BF16)
            nc.vector.memset(onecol[:], 1.0)
            ones1x128 = cst.tile([1, P], BF16)
            nc.vector.tensor_copy(
                ones1x128[:], onecol[0:1, :].broadcast_to([1, P]))
            of32 = cst.tile([65, 64], F32)
            nc.vector.memset(of32[64:65, :], 1.0)
            ones65r = cst.tile([65, 64], F32R)
            nc.vector.tensor_copy(ones65r[64:65, :], of32[64:65, :])

            # ---------- big SBUF residents ----------
            for ko in range(1, KO):
                dma2(xT[ko], xT_v[:, ko, :])
            outT = bigio.tile([P, KO, S], BF16, tag="outT")
            wp_sb = bigio.tile([P, KO, D], BF16, tag="wp")
            wv_sb = bigio.tile([P, KO, D], BF16, tag="wv")
            load_wqk(1)
            load_wqk(NPAIRS + 1)
            for ko in range(KO):
                dma2(wv_sb[:, ko, :], wvT_v[:, ko, :])

            v_sb = vpool.tile([P, ST, H * (HD + 1)], BF16)
            v_hview = v_sb[:].rearrange("p st (h c) -> p st h c", c=HD + 1)
            nc.vector.tensor_copy(
                v_hview[:, :, :, HD:HD + 1],
                onecol[:, None, None, :].broadcast_to([P, ST, H, 1]))

            qk_tiles = {}    # j -> [128, 2, S] tile (0=q, 1=k)

            # ---------- QKV work quanta (emitted interleaved) ----------
            def qk_quanta(j):
                # 4 closures; each computes one (part, nn) psum group
                t = qkp.tile([P, 2, S], BF16, tag="qkt", name=f"qk{j}")
                qk_tiles[j] = t

                def quantum(part, nn):    # part 0=q (m-tile j), 1=k (8+j)
                    def go():
                        m = j if part == 0 else NPAIRS + j
                        wt = wqk_tiles[m]
                        ps = psS.tile([P, 512], F32, tag="ps", name=f"qkps{m}")
                        for ko in range(KO):
                            nc.tensor.matmul(
                                ps[:], wt[:, ko, :],
                                xT[ko][:, nn * 512:(nn + 1) * 512],
                                start=(ko == 0), stop=(ko == KO - 1))
                        nc.vector.tensor_scalar_add(
                            t[:, part, nn * 512:(nn + 1) * 512], ps[:],
                            bqk_sb[:, m:m + 1])
                    return go
                return [quantum(0, 0), quantum(0, 1),
                        quantum(1, 0), quantum(1, 1)]

            def v_quanta(nE):
                # v half nE: e_v cols 512*nE.. (heads 8nE..8nE+7), 4 quanta
                # of 2 s-tiles reading the resident weight
                def quantum(g0):
                    def go():
                        sts = [g0, g0 + 1]
                        pss = {}
                        for st in sts:
                            pss[st] = psS.tile([P, 512], F32, tag="ps",
                                               name=f"vps{nE}_{st}")
                        for ko in range(KO):
                            for st in sts:
                                nc.tensor.matmul(
                                    pss[st][:],
                                    xT[ko][:, st * P:(st + 1) * P],
                                    wv_sb[:, ko, nE * 512:(nE + 1) * 512],
                                    start=(ko == 0), stop=(ko == KO - 1))
                        for st in sts:
                            nc.vector.tensor_copy(
                                v_hview[:, st, 8 * nE:8 * (nE + 1), 0:HD],
                                pss[st][:].rearrange("p (h c) -> p h c",
                                                     c=HD))
                    return go
                return [quantum(g) for g in (0, 2, 4, 6)]

            # ---------- attention ----------
            pend = {}

            def scores_exp(j, m):
                qk_t = qk_tiles[j]
                w = S - m * P
                for hb, base in ((0, 0), (1, 64)):   # head 2j+hb
                    at = attnp.tile([P, S], BF16, tag="at",
                                    name=f"at{j}_{hb}_{m}")
                    pend[(j, hb, m)] = at
                    off = m * P
                    for cw in _score_chunks(w):
                        ps = psS.tile([P, 512], F32, tag="ps",
                                      name=f"sps{j}_{hb}_{m}")
                        nc.tensor.matmul(
                            ps[:, 0:cw],
                            qk_t[base:base + 64, 1, m * P:(m + 1) * P],
                            qk_t[base:base + 64, 0, off:off + cw],
                            start=True, stop=True)
                        nc.scalar.activation(
                            at[:, off:off + cw], ps[:, 0:cw], AF.Exp,
                            scale=0.125)
                        off += cw
                    # causal diagonal mask on the (otherwise idle) GPSIMD
                    # engine: SBUF->SBUF multiply, keeps both PE and DVE free
                    nc.gpsimd.tensor_mul(
                        at[:, m * P:(m + 1) * P], at[:, m * P:(m + 1) * P],
                        umask[:])

            def av_m(j, m, hbs=(0, 1)):
                # exact chunking: chunk n starts at max(n*512, m*128) since
                # at[:, c] for c < m*128 is causally zero and never computed
                st8 = pend[f"ps{j}"]
                for hb in hbs:
                    h = 2 * j + hb
                    at = pend[(j, hb, m)]
                    out = st8[hb][0:65]
                    for n in range(2):
                        c0 = max(n * 512, m * P)
                        c1 = (n + 1) * 512
                        if c0 >= c1:
                            continue
                        nc.tensor.matmul(
                            out[:, c0:c1],
                            v_sb[:, m, h * (HD + 1):(h + 1) * (HD + 1)],
                            at[:, c0:c1],
                            start=(m == 0), stop=(m == 4 * n + 3))

            def evict_recip(j, hbs=(0, 1)):
                # move the AV accumulators out of PSUM so the next pair's AV
                # matmuls get the PSUM slots immediately (the tensor_copy
                # rounds to f32r, which legalizes row 64 as a f32r matmul
                # operand in rb_norm)
                for hb in hbs:
                    avc = avsbp.tile([P, S], F32R, tag="avc",
                                     name=f"avc{j}_{hb}")
                    nc.vector.tensor_copy(avc[0:65],
                                          pend[f"ps{j}"][hb][0:65])
                    pend[f"avc{j}_{hb}"] = avc
                if hbs == (0, 1) or hbs == (1,):
                    del pend[f"ps{j}"]

            def rb_norm(j, hbs=(0, 1)):
                # broadcast the RAW den row with a rank-1 f32r matmul, then
                # reciprocal AFTER the broadcast on the [64,S] tile
                # (approx_fast: ~4e-6 rel; den >= exp(0) > 0 so no edge
                # cases; multi-partition so the custom-DVE op is safe, and
                # only DVE consumes it so no f32r-rounding constraint)
                for hb in hbs:
                    avc = pend[f"avc{j}_{hb}"]
                    rb32 = rbp.tile([P, S], F32, tag="rb",
                                    name=f"rb{j}_{hb}")
                    for c in range(2):
                        rps = psS.tile([P, 512], F32, tag="ps",
                                       name=f"rbps{j}_{hb}_{c}")
                        nc.tensor.matmul(
                            rps[0:64, :], ones65r[64:65, :],
                            avc[64:65, c * 512:(c + 1) * 512],
                            start=True, stop=True)
                        nc.vector.tensor_copy(
                            rb32[0:64, c * 512:(c + 1) * 512], rps[0:64, :])
                    rbr = rtp.tile([P, S], F32, tag="rt", name=f"rbr{j}_{hb}")
                    nc.vector.reciprocal_approx_fast(
                        rbr[0:64, :], rb32[0:64, :])
                    if hb == 0:
                        nc.vector.tensor_mul(
                            outT[0:64, j, :], avc[0:64, :], rbr[0:64, :])
                    else:
                        # DVE lanes cannot shift partitions: multiply to an
                        # SBUF tmp, then DMA-shift rows 0..63 -> 64..127
                        # (partition-split across two queues)
                        tmp = toddp.tile([64, S], BF16, tag="todd",
                                         name=f"todd{j}")
                        nc.vector.tensor_mul(tmp[:], avc[0:64, :],
                                             rbr[0:64, :])
                        nc.sync.dma_start(outT[64:96, j, :], tmp[0:32])
                        nc.sync.dma_start(outT[96:128, j, :], tmp[32:64])
                    del pend[f"avc{j}_{hb}"]

            def load_wp(half):
                # projection weights prefetched mid-attention; resident wp
                # kills the tail's DMA waits
                for ko in range(4 * half, 4 * half + 4):
                    dma2(wp_sb[:, ko, :], wpT_v[:, ko, :])

            # ---------- output projection ----------
            def proj_group(g, split_tail=None, ysplit=2):
                # group g: 2 s-tiles x one nE half; ko ascending so the
                # pair-7-dependent ko=7 comes last
                nE, sts = g % 2, [2 * (g // 2), 2 * (g // 2) + 1]
                pss = {st: psS.tile([P, 512], F32, tag="ps",
                                    name=f"yps{st}_{nE}") for st in sts}
                for ko in range(7):
                    for st in sts:
                        nc.tensor.matmul(
                            pss[st][:], outT[:, ko, st * P:(st + 1) * P],
                            wp_sb[:, ko, nE * 512:(nE + 1) * 512],
                            start=(ko == 0), stop=False)
                if split_tail is not None:
                    split_tail()
                for st in sts:
                    nc.tensor.matmul(
                        pss[st][:], outT[:, 7, st * P:(st + 1) * P],
                        wp_sb[:, 7, nE * 512:(nE + 1) * 512],
                        start=False, stop=False)
                    nc.tensor.matmul(
                        pss[st][:], ones1x128[:],
                        beff_sb[:, nE * 512:(nE + 1) * 512],
                        start=False, stop=True)
                    ystg = ystgp.tile([P, 512], F32, tag="ystg",
                                      name=f"ystg{st}_{nE}")
                    nc.vector.tensor_copy(ystg[:], pss[st][:])
                    yd = y_d[st * P:(st + 1) * P, nE * 512:(nE + 1) * 512]
                    rows = P // ysplit
                    for q in range(ysplit):
                        nc.sync.dma_start(yd[rows * q:rows * (q + 1)],
                                          ystg[rows * q:rows * (q + 1)])

            # ---------- interleaved emission ----------
            # software pipeline: scores/exp for pair j+1 are emitted during
            # pair j's AV; QKV quanta fill the remaining PE slack
            for q in qk_quanta(0):
                q()
            for q in qk_quanta(1):
                q()
            load_wqk(2)
            load_wqk(NPAIRS + 2)
            vq0 = v_quanta(0)
            for g in range(4):
                vq0[g]()
                scores_exp(0, 2 * g)
                scores_exp(0, 2 * g + 1)

            vwork = []
            for j in range(NPAIRS - 1):
                work = []
                if j + 2 < NPAIRS:
                    if j > 0:
                        work.append(lambda m=j + 2: load_wqk(m))
                        work.append(lambda m=NPAIRS + j + 2: load_wqk(m))
                    work.extend(qk_quanta(j + 2))
                if j == 1:
                    vwork = v_quanta(1)
                if j in (1, 2):
                    work.append(vwork.pop(0))
                    work.append(vwork.pop(0))
                if j in (2, 3):
                    work.append(lambda h=j - 2: load_wp(h))
                for m in range(ST):
                    scores_exp(j + 1, m)
                    if m == 4 and j > 0:
                        rb_norm(j - 1)
                    if m == 0:
                        # psAV handoff: the slot waits on the previous
                        # pair's DVE eviction, so the first AV matmuls are
                        # deferred one m-step to keep the PE stream dense
                        pend[f"ps{j}"] = [
                            psAV.tile([P, S], F32, tag="av",
                                      name=f"av{j}_{hb}") for hb in range(2)]
                    elif m == 1:
                        av_m(j, 0)
                        av_m(j, 1)
                    else:
                        av_m(j, m)
                    if m % 2 == 1 and work:
                        work.pop(0)()
                        if len(work) > (ST - 1 - m) // 2:
                            work.pop(0)()
                while work:
                    work.pop(0)()
                evict_recip(j)

            # pair 7: head-major AV so head 14's evict/recip/normalize chain
            # overlaps head 15's AV matmuls
            j = NPAIRS - 1
            pend[f"ps{j}"] = [psAV.tile([P, S], F32, tag="av",
                                        name=f"av{j}_{hb}") for hb in range(2)]
            for m in range(ST):
                av_m(j, m, hbs=(0,))
                if m == 3:
                    rb_norm(j - 1)
            evict_recip(j, hbs=(0,))
            for m in range(ST):
                av_m(j, m, hbs=(1,))
                if m == 3:
                    rb_norm(j, hbs=(0,))
            evict_recip(j, hbs=(1,))
            # first proj group's ko 0..6 run while pair 7 head 15 normalizes
            proj_group(0, split_tail=lambda: rb_norm(j, hbs=(1,)))
            for g in range(1, 2 * (ST // 2)):
                proj_group(g, ysplit=(4 if g == 2 * (ST // 2) - 1 else 2))

    nc.compile()
    return nc


def kernel(x, w_attn, b_attn, w_proj, b_proj):
    import concourse.bass_utils as bass_utils
    import ml_dtypes

    if "nc" not in _CACHE:
        _CACHE["nc"] = _build()
    nc = _CACHE["nc"]

    BF = ml_dtypes.bfloat16
    x = np.asarray(x, dtype=np.float32)
    w_attn = np.asarray(w_attn, dtype=np.float32)
    b_attn = np.asarray(b_attn, dtype=np.float32)
    w_proj = np.asarray(w_proj, dtype=np.float32)
    b_proj = np.asarray(b_proj, dtype=np.float32)

    xT = np.ascontiguousarray(
        np.transpose(x, (0, 2, 1))).astype(BF)                   # [B, D, S]
    wqkT = w_attn[:2 * D].T                                      # [D, 2D]
    # [p, m, ko, c] relayout so each m-tile weight DMA reads 2KB lines
    wqk2 = np.ascontiguousarray(
        wqkT.reshape(KO, P, MT, P).transpose(1, 2, 0, 3)).astype(BF)
    wvT = np.ascontiguousarray(w_attn[2 * D:].T).astype(BF)      # [D, D]
    wpT = np.ascontiguousarray(w_proj.T).astype(BF)              # [D, D]
    bqk = np.ascontiguousarray(b_attn[:2 * D])
    bv = b_attn[2 * D:]
    beff = (b_proj.astype(np.float64)
            + w_proj.astype(np.float64) @ bv.astype(np.float64)
            ).astype(BF)
    umask = np.triu(np.ones((P, P), dtype=np.float32)).astype(BF)  # f >= p

    in_maps = [
        dict(xT=xT[b], wqk2=wqk2, wvT=wvT, wpT=wpT, bqk=bqk, beff=beff,
             umask=umask)
        for b in range(B)
    ]
    res = bass_utils.run_bass_kernel_spmd(
        nc, in_maps, core_ids=list(range(NCORES)), trace=TRACE)
    if TRACE:
        _CACHE["exec_time_ns"] = res.exec_time_ns
        _CACHE["trace"] = res.instructions_and_trace
    return np.stack([res.results[b]["y"] for b in range(B)], axis=0)


# revision 38
# speedup vs baseline: 1.4887x; 1.0221x over previous
"""Causal self-attention on 8 TRN2 NeuronCores, batch-data-parallel (one batch
element per core).

Layout strategy (per core, S=1024, D=1024, H=16, hd=64):
  - Host pre-transposes x -> xT [D,S], weights -> [in_dim, out_dim], and
    pre-rounds all matmul operands to bf16 (PE runs bf16 at 1 col/cycle like
    fp32r but at about half the power -> far less HAM clock-gate throttling;
    DVE gets 2x on 16-bit; DMA volume halves).  wqk is additionally
    host-relayouted to [p, m, ko, c] so each m-tile's weight DMA reads 2KB
    contiguous lines (the [D,2D] layout gave 256B lines at ~25% DMA
    efficiency).  All bulk DMAs use 2KB lines and are partition-split in two
    so two queues (~21 GB/s each) carry every tile.
  - qk projection produces q,k transposed ([e,s]) per head-pair: lhsT = wqkT
    tiles, rhs = xT.  Head h lives at partitions 64*(h%2)..+64.
  - v natural [s,e]: lhsT = xT tiles, rhs = resident wvT; stored interleaved
    with a ones column per head (65 cols/head) so the AV matmul's PSUM row
    64 is the softmax denominator (rowsum of unnormalized attn).  Odd heads'
    normalized output is DMA-shifted to outT rows 64..127 (partition-split
    across two queues; lane engines cannot cross partitions).
  - scoresT [sk,sq] per head-pair via K=64 matmuls; exp on ACT (scale=1/8
    folded in, bf16 out); the causal diagonal mask is a multiplicative
    [128,128] tensor_mul on the otherwise-idle GPSIMD engine (keeps PE and
    DVE free).  Fully-masked tiles are never computed.
  - AV: chunks start exactly at the causal boundary (at[:, c<128m] is never
    written); accumulated m-major into [128,S] PSUM tiles; normalization via
    approx-reciprocal of the den row (bitcast to f32r in place) + PE rank-1
    broadcast + DVE multiply.
  - proj: y[s,e] with lhsT = outT tiles, rhs = resident wpT (prefetched
    mid-attention) + rank-1 bias term (beff = b_proj + W_proj @ b_v).
  - software pipeline: scores/exp for pair j+1 are emitted during pair j's
    AV so the PE stream stays dense while ACT drains exp; QKV matmul quanta
    fill the remaining PE slack.  Pair 7 runs its AV head-major so the
    evict/recip/normalize chain of head 14 overlaps head 15's AV, and the
    first projection group's ko 0..6 matmuls are emitted before pair 7's
    normalize so the PE never waits on the tail chain.
"""

import numpy as np

B, S, D, H = 8, 1024, 1024, 16
HD = D // H          # 64
P = 128
NCORES = 8
KO = D // P          # 8 contraction tiles over d
MT = (2 * D) // P    # 16 m-tiles for q,k
ST = S // P          # 8 s-tiles
NPAIRS = H // 2      # 8 head pairs

_CACHE = {}
TRACE = False        # set by test harness to collect an NTFF profile


def _score_chunks(w):
    # split w into pieces <=512 (PSUM bank limit); bf16 streams at full rate
    # at any width so no >=256 constraint
    table = {1024: [512, 512], 896: [512, 384], 768: [512, 256],
             640: [384, 256], 512: [512], 384: [384], 256: [256], 128: [128]}
    return table[w]


def _build():
    import concourse.tile as tile
    from concourse import bacc, mybir

    F32R = mybir.dt.float32r
    F32 = mybir.dt.float32
    BF16 = mybir.dt.bfloat16
    AF = mybir.ActivationFunctionType

    nc = bacc.Bacc("TRN2", target_bir_lowering=False, debug=False,
                   num_devices=NCORES)
    xT_d = nc.dram_tensor("xT", [D, S], BF16, kind="ExternalInput").ap()
    wqk_d = nc.dram_tensor("wqk2", [P, MT, KO, P], BF16,
                           kind="ExternalInput").ap()
    wvT_d = nc.dram_tensor("wvT", [D, D], BF16, kind="ExternalInput").ap()
    wpT_d = nc.dram_tensor("wpT", [D, D], BF16, kind="ExternalInput").ap()
    bqk_d = nc.dram_tensor("bqk", [2 * D], F32, kind="ExternalInput").ap()
    beff_d = nc.dram_tensor("beff", [D], BF16, kind="ExternalInput").ap()
    umask_d = nc.dram_tensor("umask", [P, P], BF16, kind="ExternalInput").ap()
    y_d = nc.dram_tensor("y", [S, D], F32, kind="ExternalOutput").ap()

    wvT_v = wvT_d.rearrange("(ko p) e -> p ko e", p=P)
    wpT_v = wpT_d.rearrange("(ko p) e -> p ko e", p=P)
    xT_v = xT_d.rearrange("(ko p) s -> p ko s", p=P)

    with tile.TileContext(nc) as tc:
        with (
            tc.tile_pool(name="bigio", bufs=1) as bigio,
            tc.tile_pool(name="qkp", bufs=3) as qkp,
            tc.tile_pool(name="vp", bufs=1) as vpool,
            tc.tile_pool(name="wqk", bufs=4) as wqkp,
            tc.tile_pool(name="attn", bufs=20) as attnp,
            tc.tile_pool(name="rt", bufs=2) as rtp,
            tc.tile_pool(name="rb", bufs=2) as rbp,
            tc.tile_pool(name="todd", bufs=2) as toddp,
            tc.tile_pool(name="ystg", bufs=2) as ystgp,
            tc.tile_pool(name="avsb", bufs=2) as avsbp,
            tc.tile_pool(name="cst", bufs=1) as cst,
            tc.tile_pool(name="psS", bufs=4, space="PSUM") as psS,
            tc.tile_pool(name="psAV", bufs=2, space="PSUM") as psAV,
        ):
            def dma2(dst, src):
                # partition-split DMA: two queues per tile, 2KB lines
                nc.sync.dma_start(dst[0:64], src[0:64])
                nc.sync.dma_start(dst[64:128], src[64:128])

            # first-needed tiles go first: the earliest DMA queues start
            # ~3us before the bulk rings, so the first matmul's operands
            # ride them
            wqk_tiles = {}

            def load_wqk(m):
                wt = wqkp.tile([P, KO, P], BF16, tag="wqk", name=f"wqk{m}")
                nc.sync.dma_start(wt[:], wqk_d[:, m, :, :])
                wqk_tiles[m] = wt

            xT = [bigio.tile([P, S], BF16, tag=f"xT{ko}", name=f"xT{ko}")
                  for ko in range(KO)]
            load_wqk(0)
            load_wqk(NPAIRS)
            dma2(xT[0], xT_v[:, 0, :])

            # ---------- constants ----------
            umask = cst.tile([P, P], BF16)
            nc.sync.dma_start(umask[:], umask_d)
            bqk_sb = cst.tile([P, MT], F32)
            nc.sync.dma_start(bqk_sb[:], bqk_d.rearrange("(m p) -> p m", p=P))
            beff_sb = cst.tile([1, D], BF16)
            nc.sync.dma_start(beff_sb[:], beff_d[None, :])
            onecol = cst.tile([P, 1], BF16)
            nc.vector.memset(onecol[:], 1.0)
            ones1x128 = cst.tile([1, P], BF16)
            nc.vector.tensor_copy(
                ones1x128[:], onecol[0:1, :].broadcast_to([1, P]))
            of32 = cst.tile([65, 64], F32)
            nc.vector.memset(of32[64:65, :], 1.0)
            ones65r = cst.tile([65, 64], F32R)
            nc.vector.tensor_copy(ones65r[64:65, :], of32[64:65, :])

            # ---------- big SBUF residents ----------
            for ko in range(1, KO):
                dma2(xT[ko], xT_v[:, ko, :])
            outT = bigio.tile([P, KO, S], BF16, tag="outT")
            wp_sb = bigio.tile([P, KO, D], BF16, tag="wp")
            wv_sb = bigio.tile([P, KO, D], BF16, tag="wv")
            load_wqk(1)
            load_wqk(NPAIRS + 1)
            for ko in range(KO):
                dma2(wv_sb[:, ko, :], wvT_v[:, ko, :])

            v_sb = vpool.tile([P, ST, H * (HD + 1)], BF16)
            v_hview = v_sb[:].rearrange("p st (h c) -> p st h c", c=HD + 1)
            nc.vector.tensor_copy(
                v_hview[:, :, :, HD:HD + 1],
                onecol[:, None, None, :].broadcast_to([P, ST, H, 1]))

            qk_tiles = {}    # j -> [128, 2, S] tile (0=q, 1=k)

            # ---------- QKV work quanta (emitted interleaved) ----------
            def qk_quanta(j):
                # 4 closures; each computes one (part, nn) psum group
                t = qkp.tile([P, 2, S], BF16, tag="qkt", name=f"qk{j}")
                qk_tiles[j] = t

                def quantum(part, nn):    # part 0=q (m-tile j), 1=k (8+j)
                    def go():
                        m = j if part == 0 else NPAIRS + j
                        wt = wqk_tiles[m]
                        ps = psS.tile([P, 512], F32, tag="ps", name=f"qkps{m}")
                        for ko in range(KO):
                            nc.tensor.matmul(
                                ps[:], wt[:, ko, :],
                                xT[ko][:, nn * 512:(nn + 1) * 512],
                                start=(ko == 0), stop=(ko == KO - 1))
                        nc.vector.tensor_scalar_add(
                            t[:, part, nn * 512:(nn + 1) * 512], ps[:],
                            bqk_sb[:, m:m + 1])
                    return go
                return [quantum(0, 0), quantum(0, 1),
                        quantum(1, 0), quantum(1, 1)]

            def v_quanta(nE):
                # v half nE: e_v cols 512*nE.. (heads 8nE..8nE+7), 4 quanta
                # of 2 s-tiles reading the resident weight
                def quantum(g0):
                    def go():
                        sts = [g0, g0 + 1]
                        pss = {}
                        for st in sts:
                            pss[st] = psS.tile([P, 512], F32, tag="ps",
                                               name=f"vps{nE}_{st}")
                        for ko in range(KO):
                            for st in sts:
                                nc.tensor.matmul(
                                    pss[st][:],
                                    xT[ko][:, st * P:(st + 1) * P],
                                    wv_sb[:, ko, nE * 512:(nE + 1) * 512],
                                    start=(ko == 0), stop=(ko == KO - 1))
                        for st in sts:
                            nc.vector.tensor_copy(
                                v_hview[:, st, 8 * nE:8 * (nE + 1), 0:HD],
                                pss[st][:].rearrange("p (h c) -> p h c",
                                                     c=HD))
                    return go
                return [quantum(g) for g in (0, 2, 4, 6)]

            # ---------- attention ----------
            pend = {}

            def scores_exp(j, m):
                qk_t = qk_tiles[j]
                w = S - m * P
                for hb, base in ((0, 0), (1, 64)):   # head 2j+hb
                    at = attnp.tile([P, S], BF16, tag="at",
                                    name=f"at{j}_{hb}_{m}")
                    pend[(j, hb, m)] = at
                    off = m * P
                    for cw in _score_chunks(w):
                        ps = psS.tile([P, 512], F32, tag="ps",
                                      name=f"sps{j}_{hb}_{m}")
                        nc.tensor.matmul(
                            ps[:, 0:cw],
                            qk_t[base:base + 64, 1, m * P:(m + 1) * P],
                            qk_t[base:base + 64, 0, off:off + cw],
                            start=True, stop=True)
                        nc.scalar.activation(
                            at[:, off:off + cw], ps[:, 0:cw], AF.Exp,
                            scale=0.125)
                        off += cw
                    # causal diagonal mask on the (otherwise idle) GPSIMD
                    # engine: SBUF->SBUF multiply, keeps both PE and DVE free
                    nc.gpsimd.tensor_mul(
                        at[:, m * P:(m + 1) * P], at[:, m * P:(m + 1) * P],
                        umask[:])

            def av_m(j, m, hbs=(0, 1)):
                # exact chunking: chunk n starts at max(n*512, m*128) since
                # at[:, c] for c < m*128 is causally zero and never computed
                st8 = pend[f"ps{j}"]
                for hb in hbs:
                    h = 2 * j + hb
                    at = pend[(j, hb, m)]
                    out = st8[hb][0:65]
                    for n in range(2):
                        c0 = max(n * 512, m * P)
                        c1 = (n + 1) * 512
                        if c0 >= c1:
                            continue
                        nc.tensor.matmul(
                            out[:, c0:c1],
                            v_sb[:, m, h * (HD + 1):(h + 1) * (HD + 1)],
                            at[:, c0:c1],
                            start=(m == 0), stop=(m == 4 * n + 3))

            def evict_recip(j, hbs=(0, 1)):
                # move the AV accumulators out of PSUM so the next pair's AV
                # matmuls get the PSUM slots immediately (the tensor_copy
                # rounds to f32r, which legalizes row 64 as a f32r matmul
                # operand in rb_norm)
                for hb in hbs:
                    avc = avsbp.tile([P, S], F32R, tag="avc",
                                     name=f"avc{j}_{hb}")
                    nc.vector.tensor_copy(avc[0:65],
                                          pend[f"ps{j}"][hb][0:65])
                    pend[f"avc{j}_{hb}"] = avc
                if hbs == (0, 1) or hbs == (1,):
                    del pend[f"ps{j}"]

            def rb_norm(j, hbs=(0, 1), tsplit=2):
                # broadcast the RAW den row with a rank-1 f32r matmul, then
                # reciprocal AFTER the broadcast on the [64,S] tile
                # (approx_fast: ~4e-6 rel; den >= exp(0) > 0 so no edge
                # cases; multi-partition so the custom-DVE op is safe, and
                # only DVE consumes it so no f32r-rounding constraint)
                for hb in hbs:
                    avc = pend[f"avc{j}_{hb}"]
                    rb32 = rbp.tile([P, S], F32, tag="rb",
                                    name=f"rb{j}_{hb}")
                    for c in range(2):
                        rps = psS.tile([P, 512], F32, tag="ps",
                                       name=f"rbps{j}_{hb}_{c}")
                        nc.tensor.matmul(
                            rps[0:64, :], ones65r[64:65, :],
                            avc[64:65, c * 512:(c + 1) * 512],
                            start=True, stop=True)
                        nc.vector.tensor_copy(
                            rb32[0:64, c * 512:(c + 1) * 512], rps[0:64, :])
                    rbr = rtp.tile([P, S], F32, tag="rt", name=f"rbr{j}_{hb}")
                    nc.vector.reciprocal_approx_fast(
                        rbr[0:64, :], rb32[0:64, :])
                    if hb == 0:
                        nc.vector.tensor_mul(
                            outT[0:64, j, :], avc[0:64, :], rbr[0:64, :])
                    else:
                        # DVE lanes cannot shift partitions: multiply to an
                        # SBUF tmp, then DMA-shift rows 0..63 -> 64..127
                        # (partition-split across two queues)
                        tmp = toddp.tile([64, S], BF16, tag="todd",
                                         name=f"todd{j}")
                        nc.vector.tensor_mul(tmp[:], avc[0:64, :],
                                             rbr[0:64, :])
                        rws = 64 // tsplit
                        for q in range(tsplit):
                            nc.sync.dma_start(
                                outT[64 + rws * q:64 + rws * (q + 1), j, :],
                                tmp[rws * q:rws * (q + 1)])
                    del pend[f"avc{j}_{hb}"]

            def load_wp(half):
                # projection weights prefetched mid-attention; resident wp
                # kills the tail's DMA waits
                for ko in range(4 * half, 4 * half + 4):
                    dma2(wp_sb[:, ko, :], wpT_v[:, ko, :])

            # ---------- output projection ----------
            def proj_group(g, split_tail=None, ysplit=2, ko7_split=False):
                # group g: 2 s-tiles x one nE half; ko ascending so the
                # pair-7-dependent ko=7 comes last
                nE, sts = g % 2, [2 * (g // 2), 2 * (g // 2) + 1]
                pss = {st: psS.tile([P, 512], F32, tag="ps",
                                    name=f"yps{st}_{nE}") for st in sts}
                for ko in range(7):
                    for st in sts:
                        nc.tensor.matmul(
                            pss[st][:], outT[:, ko, st * P:(st + 1) * P],
                            wp_sb[:, ko, nE * 512:(nE + 1) * 512],
                            start=(ko == 0), stop=False)
                if ko7_split:
                    # even-head half of ko7 (K=64) runs before the odd-head
                    # DMA shift lands
                    for st in sts:
                        nc.tensor.matmul(
                            pss[st][:], outT[0:64, 7, st * P:(st + 1) * P],
                            wp_sb[0:64, 7, nE * 512:(nE + 1) * 512],
                            start=False, stop=False)
                if split_tail is not None:
                    split_tail()
                for st in sts:
                    if ko7_split:
                        nc.tensor.matmul(
                            pss[st][:],
                            outT[64:128, 7, st * P:(st + 1) * P],
                            wp_sb[64:128, 7, nE * 512:(nE + 1) * 512],
                            start=False, stop=False)
                    else:
                        nc.tensor.matmul(
                            pss[st][:], outT[:, 7, st * P:(st + 1) * P],
                            wp_sb[:, 7, nE * 512:(nE + 1) * 512],
                            start=False, stop=False)
                    nc.tensor.matmul(
                        pss[st][:], ones1x128[:],
                        beff_sb[:, nE * 512:(nE + 1) * 512],
                        start=False, stop=True)
                    ystg = ystgp.tile([P, 512], F32, tag="ystg",
                                      name=f"ystg{st}_{nE}")
                    nc.vector.tensor_copy(ystg[:], pss[st][:])
                    yd = y_d[st * P:(st + 1) * P, nE * 512:(nE + 1) * 512]
                    rows = P // ysplit
                    for q in range(ysplit):
                        nc.sync.dma_start(yd[rows * q:rows * (q + 1)],
                                          ystg[rows * q:rows * (q + 1)])

            # ---------- interleaved emission ----------
            # software pipeline: scores/exp for pair j+1 are emitted during
            # pair j's AV; QKV quanta fill the remaining PE slack
            for q in qk_quanta(0):
                q()
            for q in qk_quanta(1):
                q()
            load_wqk(2)
            load_wqk(NPAIRS + 2)
            vq0 = v_quanta(0)
            for g in range(4):
                vq0[g]()
                scores_exp(0, 2 * g)
                scores_exp(0, 2 * g + 1)

            vwork = []
            for j in range(NPAIRS - 1):
                work = []
                if j + 2 < NPAIRS:
                    if j > 0:
                        work.append(lambda m=j + 2: load_wqk(m))
                        work.append(lambda m=NPAIRS + j + 2: load_wqk(m))
                    work.extend(qk_quanta(j + 2))
                if j == 1:
                    vwork = v_quanta(1)
                if j in (1, 2):
                    work.append(vwork.pop(0))
                    work.append(vwork.pop(0))
                if j in (2, 3):
                    work.append(lambda h=j - 2: load_wp(h))
                for m in range(ST):
                    if j < NPAIRS - 2 or m < 4:
                        # pair 7's m>=4 scores move into the tail section so
                        # ACT load is balanced where the PE thins out
                        scores_exp(j + 1, m)
                    if m == 4 and j > 0:
                        rb_norm(j - 1, hbs=(0,))
                    if m == 6 and j > 0:
                        rb_norm(j - 1, hbs=(1,))
                    if m == 0:
                        # psAV handoff: the slot waits on the previous
                        # pair's DVE eviction, so the first AV matmuls are
                        # deferred one m-step to keep the PE stream dense
                        pend[f"ps{j}"] = [
                            psAV.tile([P, S], F32, tag="av",
                                      name=f"av{j}_{hb}") for hb in range(2)]
                    elif m == 1:
                        av_m(j, 0)
                        av_m(j, 1)
                    elif m == ST - 1:
                        # evict head 0 while head 1's last AV matmul runs
                        av_m(j, m, hbs=(0,))
                        evict_recip(j, hbs=(0,))
                        av_m(j, m, hbs=(1,))
                    else:
                        av_m(j, m)
                    if m % 2 == 1 and work:
                        work.pop(0)()
                        if len(work) > (ST - 1 - m) // 2:
                            work.pop(0)()
                while work:
                    work.pop(0)()
                evict_recip(j, hbs=(1,))

            # pair 7: head-major AV so head 14's evict/recip/normalize chain
            # overlaps head 15's AV matmuls; pair 7's m>=4 scores/exp land
            # here where ACT is otherwise idle
            j = NPAIRS - 1
            pend[f"ps{j}"] = [psAV.tile([P, S], F32, tag="av",
                                        name=f"av{j}_{hb}") for hb in range(2)]
            for m in range(ST):
                if m < 4:
                    scores_exp(j, m + 4)
                av_m(j, m, hbs=(0,))
                if m == 2:
                    rb_norm(j - 1, hbs=(0,))
                if m == 4:
                    rb_norm(j - 1, hbs=(1,))
            evict_recip(j, hbs=(0,))
            for m in range(ST):
                av_m(j, m, hbs=(1,))
                if m == 3:
                    rb_norm(j, hbs=(0,))
            evict_recip(j, hbs=(1,))
            # first proj group's ko 0..6 (and the even half of ko7) run while
            # pair 7 head 15 normalizes; only the odd half waits on the shift
            proj_group(0, split_tail=lambda: rb_norm(j, hbs=(1,), tsplit=4),
                       ko7_split=True)
            for g in range(1, 2 * (ST // 2)):
                proj_group(g, ysplit=(4 if g == 2 * (ST // 2) - 1 else 2))

    nc.compile()
    return nc


def kernel(x, w_attn, b_attn, w_proj, b_proj):
    import concourse.bass_utils as bass_utils
    import ml_dtypes

    if "nc" not in _CACHE:
        _CACHE["nc"] = _build()
    nc = _CACHE["nc"]

    BF = ml_dtypes.bfloat16
    x = np.asarray(x, dtype=np.float32)
    w_attn = np.asarray(w_attn, dtype=np.float32)
    b_attn = np.asarray(b_attn, dtype=np.float32)
    w_proj = np.asarray(w_proj, dtype=np.float32)
    b_proj = np.asarray(b_proj, dtype=np.float32)

    xT = np.ascontiguousarray(
        np.transpose(x, (0, 2, 1))).astype(BF)                   # [B, D, S]
    wqkT = w_attn[:2 * D].T                                      # [D, 2D]
    # [p, m, ko, c] relayout so each m-tile weight DMA reads 2KB lines
    wqk2 = np.ascontiguousarray(
        wqkT.reshape(KO, P, MT, P).transpose(1, 2, 0, 3)).astype(BF)
    wvT = np.ascontiguousarray(w_attn[2 * D:].T).astype(BF)      # [D, D]
    wpT = np.ascontiguousarray(w_proj.T).astype(BF)              # [D, D]
    bqk = np.ascontiguousarray(b_attn[:2 * D])
    bv = b_attn[2 * D:]
    beff = (b_proj.astype(np.float64)
            + w_proj.astype(np.float64) @ bv.astype(np.float64)
            ).astype(BF)
    umask = np.triu(np.ones((P, P), dtype=np.float32)).astype(BF)  # f >= p

    in_maps = [
        dict(xT=xT[b], wqk2=wqk2, wvT=wvT, wpT=wpT, bqk=bqk, beff=beff,
             umask=umask)
        for b in range(B)
    ]
    res = bass_utils.run_bass_kernel_spmd(
        nc, in_maps, core_ids=list(range(NCORES)), trace=TRACE)
    if TRACE:
        _CACHE["exec_time_ns"] = res.exec_time_ns
        _CACHE["trace"] = res.instructions_and_trace
    return np.stack([res.results[b]["y"] for b in range(B)], axis=0)


# revision 45
# speedup vs baseline: 1.5140x; 1.0170x over previous
"""Causal self-attention on 8 TRN2 NeuronCores, batch-data-parallel (one batch
element per core).

Layout strategy (per core, S=1024, D=1024, H=16, hd=64):
  - Host pre-transposes x -> xT [D,S], weights -> [in_dim, out_dim], and
    pre-rounds all matmul operands to bf16 (PE runs bf16 at 1 col/cycle like
    fp32r but at about half the power -> far less HAM clock-gate throttling;
    DVE gets 2x on 16-bit; DMA volume halves).  wqk is additionally
    host-relayouted to [p, m, ko, c] so each m-tile's weight DMA reads 2KB
    contiguous lines (the [D,2D] layout gave 256B lines at ~25% DMA
    efficiency).  All bulk DMAs use 2KB lines and are partition-split in two
    so two queues (~21 GB/s each) carry every tile.
  - qk projection produces q,k transposed ([e,s]) per head-pair: lhsT = wqkT
    tiles, rhs = xT.  Head h lives at partitions 64*(h%2)..+64.
  - v natural [s,e]: lhsT = xT tiles, rhs = resident wvT; stored interleaved
    with a ones column per head (65 cols/head) so the AV matmul's PSUM row
    64 is the softmax denominator (rowsum of unnormalized attn).  Odd heads'
    normalized output is DMA-shifted to outT rows 64..127 (partition-split
    across two queues; lane engines cannot cross partitions).
  - scoresT [sk,sq] per head-pair via K=64 matmuls; exp on ACT (scale=1/8
    folded in, bf16 out); the causal diagonal mask is a multiplicative
    [128,128] tensor_mul on the otherwise-idle GPSIMD engine (keeps PE and
    DVE free).  Fully-masked tiles are never computed.
  - AV: chunks start exactly at the causal boundary (at[:, c<128m] is never
    written); accumulated m-major into [128,S] PSUM tiles; normalization via
    approx-reciprocal of the den row (bitcast to f32r in place) + PE rank-1
    broadcast + DVE multiply.
  - proj: y[s,e] with lhsT = outT tiles, rhs = resident wpT (prefetched
    mid-attention) + rank-1 bias term (beff = b_proj + W_proj @ b_v).
  - software pipeline: scores/exp for pair j+1 are emitted during pair j's
    AV so the PE stream stays dense while ACT drains exp; QKV matmul quanta
    fill the remaining PE slack.  Pair 7 runs its AV head-major so the
    evict/recip/normalize chain of head 14 overlaps head 15's AV, and the
    first projection group's ko 0..6 matmuls are emitted before pair 7's
    normalize so the PE never waits on the tail chain.
"""

import numpy as np

B, S, D, H = 8, 1024, 1024, 16
HD = D // H          # 64
P = 128
NCORES = 8
KO = D // P          # 8 contraction tiles over d
MT = (2 * D) // P    # 16 m-tiles for q,k
ST = S // P          # 8 s-tiles
NPAIRS = H // 2      # 8 head pairs

_CACHE = {}
TRACE = False        # set by test harness to collect an NTFF profile


def _score_chunks(w):
    # split w into pieces <=512 (PSUM bank limit); bf16 streams at full rate
    # at any width so no >=256 constraint
    table = {1024: [512, 512], 896: [512, 384], 768: [512, 256],
             640: [384, 256], 512: [512], 384: [384], 256: [256], 128: [128]}
    return table[w]


def _build():
    import concourse.tile as tile
    from concourse import bacc, mybir

    F32R = mybir.dt.float32r
    F32 = mybir.dt.float32
    BF16 = mybir.dt.bfloat16
    AF = mybir.ActivationFunctionType

    nc = bacc.Bacc("TRN2", target_bir_lowering=False, debug=False,
                   num_devices=NCORES)
    xT_d = nc.dram_tensor("xT", [D, S], BF16, kind="ExternalInput").ap()
    wqk_d = nc.dram_tensor("wqk2", [P, MT, KO, P], BF16,
                           kind="ExternalInput").ap()
    wvT_d = nc.dram_tensor("wvT", [D, D], BF16, kind="ExternalInput").ap()
    wpT_d = nc.dram_tensor("wpT", [D, D], BF16, kind="ExternalInput").ap()
    bqk_d = nc.dram_tensor("bqk", [2 * D], F32, kind="ExternalInput").ap()
    beff_d = nc.dram_tensor("beff", [D], BF16, kind="ExternalInput").ap()
    umask_d = nc.dram_tensor("umask", [P, P], BF16, kind="ExternalInput").ap()
    y_d = nc.dram_tensor("y", [S, D], F32, kind="ExternalOutput").ap()

    wvT_v = wvT_d.rearrange("(ko p) e -> p ko e", p=P)
    wpT_v = wpT_d.rearrange("(ko p) e -> p ko e", p=P)
    xT_v = xT_d.rearrange("(ko p) s -> p ko s", p=P)

    with tile.TileContext(nc) as tc:
        with (
            tc.tile_pool(name="bigio", bufs=1) as bigio,
            tc.tile_pool(name="qkp", bufs=3) as qkp,
            tc.tile_pool(name="vp", bufs=1) as vpool,
            tc.tile_pool(name="wqk", bufs=4) as wqkp,
            tc.tile_pool(name="attn", bufs=20) as attnp,
            tc.tile_pool(name="rt", bufs=2) as rtp,
            tc.tile_pool(name="rb", bufs=2) as rbp,
            tc.tile_pool(name="todd", bufs=2) as toddp,
            tc.tile_pool(name="ystg", bufs=2) as ystgp,
            tc.tile_pool(name="avsb", bufs=2) as avsbp,
            tc.tile_pool(name="cst", bufs=1) as cst,
            tc.tile_pool(name="psS", bufs=4, space="PSUM") as psS,
            tc.tile_pool(name="psAV", bufs=2, space="PSUM") as psAV,
        ):
            def dma2(dst, src):
                # partition-split DMA: two queues per tile, 2KB lines
                nc.sync.dma_start(dst[0:64], src[0:64])
                nc.sync.dma_start(dst[64:128], src[64:128])

            # first-needed tiles go first: the earliest DMA queues start
            # ~3us before the bulk rings, so the first matmul's operands
            # ride them
            wqk_tiles = {}

            def load_wqk(m):
                wt = wqkp.tile([P, KO, P], BF16, tag="wqk", name=f"wqk{m}")
                nc.sync.dma_start(wt[:], wqk_d[:, m, :, :])
                wqk_tiles[m] = wt

            xT = [bigio.tile([P, S], BF16, tag=f"xT{ko}", name=f"xT{ko}")
                  for ko in range(KO)]
            load_wqk(0)
            load_wqk(NPAIRS)
            dma2(xT[0], xT_v[:, 0, :])

            # ---------- constants ----------
            umask = cst.tile([P, P], BF16)
            nc.sync.dma_start(umask[:], umask_d)
            bqk_sb = cst.tile([P, MT], F32)
            nc.sync.dma_start(bqk_sb[:], bqk_d.rearrange("(m p) -> p m", p=P))
            beff_sb = cst.tile([1, D], BF16)
            nc.sync.dma_start(beff_sb[:], beff_d[None, :])
            onecol = cst.tile([P, 1], BF16)
            nc.vector.memset(onecol[:], 1.0)
            ones1x128 = cst.tile([1, P], BF16)
            nc.vector.tensor_copy(
                ones1x128[:], onecol[0:1, :].broadcast_to([1, P]))
            of32 = cst.tile([65, 64], F32)
            nc.vector.memset(of32[64:65, :], 1.0)
            ones65r = cst.tile([65, 64], F32R)
            nc.vector.tensor_copy(ones65r[64:65, :], of32[64:65, :])

            # ---------- big SBUF residents ----------
            for ko in range(1, KO):
                dma2(xT[ko], xT_v[:, ko, :])
            outT = bigio.tile([P, KO, S], BF16, tag="outT")
            wp_sb = bigio.tile([P, KO, D], BF16, tag="wp")
            wv_sb = bigio.tile([P, KO, D], BF16, tag="wv")
            load_wqk(1)
            load_wqk(NPAIRS + 1)
            for ko in range(KO):
                dma2(wv_sb[:, ko, :], wvT_v[:, ko, :])

            v_sb = vpool.tile([P, ST, H * (HD + 1)], BF16)
            v_hview = v_sb[:].rearrange("p st (h c) -> p st h c", c=HD + 1)
            nc.vector.tensor_copy(
                v_hview[:, :, :, HD:HD + 1],
                onecol[:, None, None, :].broadcast_to([P, ST, H, 1]))

            qk_tiles = {}    # j -> [128, 2, S] tile (0=q, 1=k)

            # ---------- QKV work quanta (emitted interleaved) ----------
            def qk_quanta(j):
                # 4 closures; each computes one (part, nn) psum group
                t = qkp.tile([P, 2, S], BF16, tag="qkt", name=f"qk{j}")
                qk_tiles[j] = t

                def quantum(part, nn):    # part 0=q (m-tile j), 1=k (8+j)
                    def go():
                        m = j if part == 0 else NPAIRS + j
                        wt = wqk_tiles[m]
                        ps = psS.tile([P, 512], F32, tag="ps", name=f"qkps{m}")
                        for ko in range(KO):
                            nc.tensor.matmul(
                                ps[:], wt[:, ko, :],
                                xT[ko][:, nn * 512:(nn + 1) * 512],
                                start=(ko == 0), stop=(ko == KO - 1))
                        nc.vector.tensor_scalar_add(
                            t[:, part, nn * 512:(nn + 1) * 512], ps[:],
                            bqk_sb[:, m:m + 1])
                    return go
                return [quantum(0, 0), quantum(0, 1),
                        quantum(1, 0), quantum(1, 1)]

            def v_quanta(nE):
                # v half nE: e_v cols 512*nE.. (heads 8nE..8nE+7), 4 quanta
                # of 2 s-tiles reading the resident weight
                def quantum(g0):
                    def go():
                        # st-sequential so only one psS slot is held at a
                        # time (two held at once collapses the scores ring
                        # distance below the ACT exp round-trip)
                        for st in (g0, g0 + 1):
                            ps = psS.tile([P, 512], F32, tag="ps",
                                          name=f"vps{nE}_{st}")
                            for ko in range(KO):
                                nc.tensor.matmul(
                                    ps[:],
                                    xT[ko][:, st * P:(st + 1) * P],
                                    wv_sb[:, ko, nE * 512:(nE + 1) * 512],
                                    start=(ko == 0), stop=(ko == KO - 1))
                            nc.vector.tensor_copy(
                                v_hview[:, st, 8 * nE:8 * (nE + 1), 0:HD],
                                ps[:].rearrange("p (h c) -> p h c", c=HD))
                    return go
                return [quantum(g) for g in (0, 2, 4, 6)]

            # ---------- attention ----------
            pend = {}

            def scores_exp(j, m):
                qk_t = qk_tiles[j]
                w = S - m * P
                for hb, base in ((0, 0), (1, 64)):   # head 2j+hb
                    at = attnp.tile([P, S], BF16, tag="at",
                                    name=f"at{j}_{hb}_{m}")
                    pend[(j, hb, m)] = at
                    off = m * P
                    for cw in _score_chunks(w):
                        ps = psS.tile([P, 512], F32, tag="ps",
                                      name=f"sps{j}_{hb}_{m}")
                        nc.tensor.matmul(
                            ps[:, 0:cw],
                            qk_t[base:base + 64, 1, m * P:(m + 1) * P],
                            qk_t[base:base + 64, 0, off:off + cw],
                            start=True, stop=True)
                        nc.scalar.activation(
                            at[:, off:off + cw], ps[:, 0:cw], AF.Exp,
                            scale=0.125)
                        off += cw
                    # causal diagonal mask on the (otherwise idle) GPSIMD
                    # engine: SBUF->SBUF multiply, keeps both PE and DVE free
                    nc.gpsimd.tensor_mul(
                        at[:, m * P:(m + 1) * P], at[:, m * P:(m + 1) * P],
                        umask[:])

            def av_m(j, m, hbs=(0, 1)):
                # exact chunking: chunk n starts at max(n*512, m*128) since
                # at[:, c] for c < m*128 is causally zero and never computed
                st8 = pend[f"ps{j}"]
                for hb in hbs:
                    h = 2 * j + hb
                    at = pend[(j, hb, m)]
                    out = st8[hb][0:65]
                    for n in range(2):
                        c0 = max(n * 512, m * P)
                        c1 = (n + 1) * 512
                        if c0 >= c1:
                            continue
                        nc.tensor.matmul(
                            out[:, c0:c1],
                            v_sb[:, m, h * (HD + 1):(h + 1) * (HD + 1)],
                            at[:, c0:c1],
                            start=(m == 0), stop=(m == 4 * n + 3))

            def cp(eng, dst, src):
                # PSUM->SBUF eviction on a selectable engine: DVE normally;
                # ACT (activation Copy) near the pair-6/7 boundary where the
                # DVE FIFO backs up and ACT has slack (pair 7's exps are
                # half-moved into the tail)
                if eng == "s":
                    nc.scalar.activation(dst, src, AF.Copy)
                else:
                    nc.vector.tensor_copy(dst, src)

            def evict_recip(j, hbs=(0, 1), eng="v"):
                # move the AV accumulators out of PSUM so the next pair's AV
                # matmuls get the PSUM slots immediately (the copy rounds to
                # f32r, which legalizes row 64 as a f32r matmul operand in
                # rb_norm)
                for hb in hbs:
                    avc = avsbp.tile([P, S], F32R, tag="avc",
                                     name=f"avc{j}_{hb}")
                    cp(eng, avc[0:65], pend[f"ps{j}"][hb][0:65])
                    pend[f"avc{j}_{hb}"] = avc
                if hbs == (0, 1) or hbs == (1,):
                    del pend[f"ps{j}"]

            def rb_norm(j, hbs=(0, 1), tsplit=2, eng="v"):
                # broadcast the RAW den row with a rank-1 f32r matmul, then
                # reciprocal AFTER the broadcast on the [64,S] tile
                # (approx_fast: ~4e-6 rel; den >= exp(0) > 0 so no edge
                # cases; multi-partition so the custom-DVE op is safe, and
                # only DVE consumes it so no f32r-rounding constraint)
                for hb in hbs:
                    avc = pend[f"avc{j}_{hb}"]
                    rb32 = rbp.tile([P, S], F32, tag="rb",
                                    name=f"rb{j}_{hb}")
                    for c in range(2):
                        rps = psS.tile([P, 512], F32, tag="ps",
                                       name=f"rbps{j}_{hb}_{c}")
                        nc.tensor.matmul(
                            rps[0:64, :], ones65r[64:65, :],
                            avc[64:65, c * 512:(c + 1) * 512],
                            start=True, stop=True)
                        cp(eng, rb32[0:64, c * 512:(c + 1) * 512],
                           rps[0:64, :])
                    rbr = rtp.tile([P, S], F32, tag="rt", name=f"rbr{j}_{hb}")
                    nc.vector.reciprocal_approx_fast(
                        rbr[0:64, :], rb32[0:64, :])
                    if hb == 0:
                        nc.vector.tensor_mul(
                            outT[0:64, j, :], avc[0:64, :], rbr[0:64, :])
                    else:
                        # DVE lanes cannot shift partitions: multiply to an
                        # SBUF tmp, then DMA-shift rows 0..63 -> 64..127
                        # (partition-split across two queues)
                        tmp = toddp.tile([64, S], BF16, tag="todd",
                                         name=f"todd{j}")
                        nc.vector.tensor_mul(tmp[:], avc[0:64, :],
                                             rbr[0:64, :])
                        rws = 64 // tsplit
                        for q in range(tsplit):
                            nc.sync.dma_start(
                                outT[64 + rws * q:64 + rws * (q + 1), j, :],
                                tmp[rws * q:rws * (q + 1)])
                    del pend[f"avc{j}_{hb}"]

            def load_wp(half):
                # projection weights prefetched mid-attention; resident wp
                # kills the tail's DMA waits
                for ko in range(4 * half, 4 * half + 4):
                    dma2(wp_sb[:, ko, :], wpT_v[:, ko, :])

            # ---------- output projection ----------
            def proj_group(g, split_tail=None, ysplit=2, ko7_split=False):
                # group g: 2 s-tiles x one nE half; ko ascending so the
                # pair-7-dependent ko=7 comes last
                nE, sts = g % 2, [2 * (g // 2), 2 * (g // 2) + 1]
                pss = {st: psS.tile([P, 512], F32, tag="ps",
                                    name=f"yps{st}_{nE}") for st in sts}
                for ko in range(7):
                    for st in sts:
                        nc.tensor.matmul(
                            pss[st][:], outT[:, ko, st * P:(st + 1) * P],
                            wp_sb[:, ko, nE * 512:(nE + 1) * 512],
                            start=(ko == 0), stop=False)
                if ko7_split:
                    # even-head half of ko7 (K=64) runs before the odd-head
                    # DMA shift lands
                    for st in sts:
                        nc.tensor.matmul(
                            pss[st][:], outT[0:64, 7, st * P:(st + 1) * P],
                            wp_sb[0:64, 7, nE * 512:(nE + 1) * 512],
                            start=False, stop=False)
                if split_tail is not None:
                    split_tail()
                for st in sts:
                    if ko7_split:
                        nc.tensor.matmul(
                            pss[st][:],
                            outT[64:128, 7, st * P:(st + 1) * P],
                            wp_sb[64:128, 7, nE * 512:(nE + 1) * 512],
                            start=False, stop=False)
                    else:
                        nc.tensor.matmul(
                            pss[st][:], outT[:, 7, st * P:(st + 1) * P],
                            wp_sb[:, 7, nE * 512:(nE + 1) * 512],
                            start=False, stop=False)
                    nc.tensor.matmul(
                        pss[st][:], ones1x128[:],
                        beff_sb[:, nE * 512:(nE + 1) * 512],
                        start=False, stop=True)
                    ystg = ystgp.tile([P, 512], F32, tag="ystg",
                                      name=f"ystg{st}_{nE}")
                    nc.vector.tensor_copy(ystg[:], pss[st][:])
                    yd = y_d[st * P:(st + 1) * P, nE * 512:(nE + 1) * 512]
                    rows = P // ysplit
                    for q in range(ysplit):
                        nc.sync.dma_start(yd[rows * q:rows * (q + 1)],
                                          ystg[rows * q:rows * (q + 1)])

            # ---------- interleaved emission ----------
            # software pipeline: scores/exp for pair j+1 are emitted during
            # pair j's AV; QKV quanta fill the remaining PE slack
            for q in qk_quanta(0):
                q()
            for q in qk_quanta(1):
                q()
            load_wqk(2)
            load_wqk(NPAIRS + 2)
            vq0 = v_quanta(0)
            for g in range(4):
                vq0[g]()
                scores_exp(0, 2 * g)
                scores_exp(0, 2 * g + 1)

            vwork = []
            for j in range(NPAIRS - 1):
                work = []
                if j + 2 < NPAIRS:
                    if j > 0:
                        work.append(lambda m=j + 2: load_wqk(m))
                        work.append(lambda m=NPAIRS + j + 2: load_wqk(m))
                    work.extend(qk_quanta(j + 2))
                if j == 1:
                    vwork = v_quanta(1)
                if j in (1, 2):
                    work.append(vwork.pop(0))
                    work.append(vwork.pop(0))
                if j in (2, 3):
                    work.append(lambda h=j - 2: load_wp(h))
                for m in range(ST):
                    if j < NPAIRS - 2 or m < 4:
                        # pair 7's m>=4 scores move into the tail section so
                        # ACT load is balanced where the PE thins out
                        scores_exp(j + 1, m)
                    if m == 4 and j > 0:
                        rb_norm(j - 1, hbs=(0,), eng="s" if j >= 6 else "v")
                    if m == 6 and j > 0:
                        rb_norm(j - 1, hbs=(1,), eng="s" if j >= 6 else "v")
                    if m == 0:
                        # psAV handoff: the slot waits on the previous
                        # pair's DVE eviction, so the first AV matmuls are
                        # deferred one m-step to keep the PE stream dense
                        pend[f"ps{j}"] = [
                            psAV.tile([P, S], F32, tag="av",
                                      name=f"av{j}_{hb}") for hb in range(2)]
                    elif m == 1:
                        av_m(j, 0)
                        av_m(j, 1)
                    elif m == ST - 1:
                        # evict head 0 while head 1's last AV matmul runs
                        av_m(j, m, hbs=(0,))
                        evict_recip(j, hbs=(0,), eng="s" if j >= 6 else "v")
                        av_m(j, m, hbs=(1,))
                    else:
                        av_m(j, m)
                    if m % 2 == 1 and work:
                        work.pop(0)()
                        if len(work) > (ST - 1 - m) // 2:
                            work.pop(0)()
                while work:
                    work.pop(0)()
                evict_recip(j, hbs=(1,), eng="s" if j >= 6 else "v")

            # pair 7: head-major AV so head 14's evict/recip/normalize chain
            # overlaps head 15's AV matmuls; pair 7's m>=4 scores/exp land
            # here where ACT is otherwise idle
            j = NPAIRS - 1
            pend[f"ps{j}"] = [psAV.tile([P, S], F32, tag="av",
                                        name=f"av{j}_{hb}") for hb in range(2)]
            for m in range(ST):
                if m < 4:
                    scores_exp(j, m + 4)
                av_m(j, m, hbs=(0,))
                if m == 2:
                    rb_norm(j - 1, hbs=(0,), eng="s")
                if m == 4:
                    rb_norm(j - 1, hbs=(1,), tsplit=4, eng="s")
            evict_recip(j, hbs=(0,), eng="s")
            for m in range(ST):
                av_m(j, m, hbs=(1,))
                if m == 3:
                    rb_norm(j, hbs=(0,), eng="s")
            evict_recip(j, hbs=(1,), eng="s")
            # first proj group's ko 0..6 (and the even half of ko7) run while
            # pair 7 head 15 normalizes; only the odd half waits on the shift
            proj_group(0, split_tail=lambda: rb_norm(j, hbs=(1,), tsplit=4,
                                                     eng="s"),
                       ko7_split=True)
            for g in range(1, 2 * (ST // 2)):
                proj_group(g, ysplit=(4 if g == 2 * (ST // 2) - 1 else 2))

    nc.compile()
    return nc


def kernel(x, w_attn, b_attn, w_proj, b_proj):
    import concourse.bass_utils as bass_utils
    import ml_dtypes

    if "nc" not in _CACHE:
        _CACHE["nc"] = _build()
    nc = _CACHE["nc"]

    BF = ml_dtypes.bfloat16
    x = np.asarray(x, dtype=np.float32)
    w_attn = np.asarray(w_attn, dtype=np.float32)
    b_attn = np.asarray(b_attn, dtype=np.float32)
    w_proj = np.asarray(w_proj, dtype=np.float32)
    b_proj = np.asarray(b_proj, dtype=np.float32)

    xT = np.ascontiguousarray(
        np.transpose(x, (0, 2, 1))).astype(BF)                   # [B, D, S]
    wqkT = w_attn[:2 * D].T                                      # [D, 2D]
    # [p, m, ko, c] relayout so each m-tile weight DMA reads 2KB lines
    wqk2 = np.ascontiguousarray(
        wqkT.reshape(KO, P, MT, P).transpose(1, 2, 0, 3)).astype(BF)
    wvT = np.ascontiguousarray(w_attn[2 * D:].T).astype(BF)      # [D, D]
    wpT = np.ascontiguousarray(w_proj.T).astype(BF)              # [D, D]
    bqk = np.ascontiguousarray(b_attn[:2 * D])
    bv = b_attn[2 * D:]
    beff = (b_proj.astype(np.float64)
            + w_proj.astype(np.float64) @ bv.astype(np.float64)
            ).astype(BF)
    umask = np.triu(np.ones((P, P), dtype=np.float32)).astype(BF)  # f >= p

    in_maps = [
        dict(xT=xT[b], wqk2=wqk2, wvT=wvT, wpT=wpT, bqk=bqk, beff=beff,
             umask=umask)
        for b in range(B)
    ]
    res = bass_utils.run_bass_kernel_spmd(
        nc, in_maps, core_ids=list(range(NCORES)), trace=TRACE)
    if TRACE:
        _CACHE["exec_time_ns"] = res.exec_time_ns
        _CACHE["trace"] = res.instructions_and_trace
    return np.stack([res.results[b]["y"] for b in range(B)], axis=0)
